# revision 25
# baseline (speedup 1.0000x reference)
"""DCNv2 (deformable conv) Trainium2 Bass kernel.

Strategy (per core, pure batch data-parallel across 8 cores):
  - x padded (+1) on host; SBUF resident per image: xs [96c, (H+2)(W+2)].
  - PE computes offset/mask 3x3 convs (9 accumulating matmuls per output row,
    pixels-on-partitions psum [128w, 27]) and per-tap 1x1 convs
    Y_k(o,r,w) = sum_c W[o,c,k] x(c,r,w) into a banded SBUF tensor
    YT [128w, (RB+6)rows, 9k, 96o] with zero row-halo.
  - DVE builds per-pixel bilinear row/col interpolation fields vy/hxm
    (integer sample-offset one-hots weighted by frac parts; mask folded
    into hxm).  Column shifts u are realized by DMA partition-shifted
    copies of vy/hxm (DMA is exempt from the partition-window rule).
  - DVE combine, per output row h and column shift u: one TT-mult with
    multi-dim APs over (i, j, ty, o) times q_u, then an XYZ tensor_reduce
    -> red_u[s, o] (s = source column).  PE merges the 7 shifted partials
    with shifted-identity matmuls accumulating psum[o, w]; ACT adds bias
    during the PSUM->SBUF copy; DMA out per band.
"""

import sys

sys.path.insert(0, "/opt/trn_rl_repo")

import numpy as np

import concourse.bacc as bacc
import concourse.bass as bass
import concourse.mybir as mybir
from concourse.tile import TileContext

F32 = mybir.dt.float32
AF = mybir.ActivationFunctionType
AL = mybir.AluOpType

C = 96
O = 96
NTAP = 9
W = 128
NCORES = 8


def build_nc(H=128, BS=2, RB=8, TYLO=-2, TYHI=2, num_devices=NCORES):
    """Build the per-core Bass program."""
    nty = TYHI - TYLO + 1
    ntx = nty
    TXLO, TXHI = TYLO, TYHI
    Hp, Wp = H + 2, W + 2
    RS = NTAP * O          # YT row stride = 864
    NROW = RB + 6          # YT band rows incl +-3 halo
    NK9 = RB * NTAP
    US = list(range(TXLO - 1, TXHI + 2))   # column shifts u = (j-1)+tx
    assert H % RB == 0

    nc = bacc.Bacc("TRN2", target_bir_lowering=False, debug=False,
                   num_devices=num_devices, dynamic_dma_scratch_size=2048)

    xp = nc.dram_tensor("xp", [BS, C, Hp * Wp], F32, kind="ExternalInput")
    wmain = nc.dram_tensor("wmain", [C, NTAP * O], F32, kind="ExternalInput")
    womb = nc.dram_tensor("womb", [C, NTAP * 27], F32, kind="ExternalInput")
    obrep = nc.dram_tensor("obrep", [W, 27], F32, kind="ExternalInput")
    ity = nc.dram_tensor("ity", [W, nty], F32, kind="ExternalInput")
    identw = nc.dram_tensor("identw", [W, W + 6], F32, kind="ExternalInput")
    biaso = nc.dram_tensor("biaso", [O, 1], F32, kind="ExternalInput")
    out = nc.dram_tensor("out", [BS, O, H * W], F32, kind="ExternalOutput")

    def sb_view(tile, offset, dims):
        return bass.AP(tensor=tile.tensor, offset=int(tile.offset) + offset,
                       ap=[list(d) for d in dims])

    with TileContext(nc) as tc:
        with (
            tc.tile_pool(name="consts", bufs=1) as cpool,
            tc.tile_pool(name="xs", bufs=1) as xpool,
            tc.tile_pool(name="yt", bufs=1) as ytpool,
            tc.tile_pool(name="fields", bufs=1) as fpool,
            tc.tile_pool(name="shf", bufs=1) as shpool,
            tc.tile_pool(name="qp", bufs=1) as qpool,
            tc.tile_pool(name="tmp", bufs=1) as tmppool,
            tc.tile_pool(name="small", bufs=4) as spool,
            tc.tile_pool(name="obuf", bufs=1) as opool,
            tc.tile_pool(name="psum_om", bufs=2, space="PSUM") as ompool,
            tc.tile_pool(name="psum_y", bufs=4, space="PSUM") as ypool,
            tc.tile_pool(name="psum_t", bufs=2, space="PSUM") as tpool,
        ):
            wmain_sb = cpool.tile([C, NTAP * O], F32)
            womb_sb = cpool.tile([C, NTAP * 27], F32)
            obrep_sb = cpool.tile([W, 27], F32)
            ity_sb = cpool.tile([W, nty], F32)
            identw_sb = cpool.tile([W, W + 6], F32)
            biaso_sb = cpool.tile([O, 1], F32)
            nc.sync.dma_start(wmain_sb[:], wmain[:])
            nc.sync.dma_start(womb_sb[:], womb[:])
            nc.sync.dma_start(obrep_sb[:], obrep[:])
            nc.sync.dma_start(ity_sb[:], ity[:])
            nc.sync.dma_start(identw_sb[:], identw[:])
            nc.sync.dma_start(biaso_sb[:], biaso[:])

            # partition-shifted field copies: value at partition s is the
            # field of destination column w = s - u; zero in the strips.
            vy_u = {}
            hxm_u = {}
            for u in US:
                if u == 0:
                    continue
                vy_u[u] = shpool.tile([W, NK9 * nty], F32, tag=f"vyu{u}", name=f"vyu{u}")
                hxm_u[u] = shpool.tile([W, NK9 * ntx], F32, tag=f"hxu{u}", name=f"hxu{u}")
                nc.vector.memset(vy_u[u][:], 0.0)
                nc.vector.memset(hxm_u[u][:], 0.0)

            for img in range(BS):
                xs = xpool.tile([C, Hp * Wp], F32, tag="xs")
                nc.sync.dma_start(
                    xs[:],
                    bass.AP(tensor=xp, offset=img * C * Hp * Wp,
                            ap=[[Hp * Wp, C], [1, Hp * Wp]]))

                for band in range(H // RB):
                    b0 = band * RB

                    # ---- offset/mask convs + raw fields -------------------
                    raw = fpool.tile([W, RB * 27], F32, tag="raw")
                    for hh in range(RB):
                        h = b0 + hh
                        ps_om = ompool.tile([W, 27], F32, tag="om")
                        for t in range(NTAP):
                            ti, tj = t // 3, t % 3
                            lhsT = sb_view(xs, (h + ti) * Wp + tj,
                                           [[Hp * Wp, C], [1, W]])
                            nc.tensor.matmul(ps_om[:], lhsT,
                                             womb_sb[:, t * 27:(t + 1) * 27],
                                             start=(t == 0), stop=(t == NTAP - 1))
                        nc.vector.tensor_add(
                            out=raw[:, hh * 27:(hh + 1) * 27],
                            in0=ps_om[:], in1=obrep_sb[:])

                    # ---- per-pixel interpolation fields -------------------
                    dyv = sb_view(raw, 0, [[RB * 27, W], [27, RB], [2, NTAP]])
                    dxv = sb_view(raw, 1, [[RB * 27, W], [27, RB], [2, NTAP]])
                    mrawv = sb_view(raw, 18, [[RB * 27, W], [27, RB], [1, NTAP]])

                    def pk2(tile):  # packed [hh][k] view matching (RB, NTAP)
                        return sb_view(tile, 0, [[NK9, W], [NTAP, RB], [1, NTAP]])

                    msk = fpool.tile([W, NK9], F32, tag="msk")
                    nc.scalar.activation(out=pk2(msk), in_=mrawv, func=AF.Sigmoid)

                    MAGIC = 12582912.0  # 1.5 * 2**23: fp32 round-to-int magic

                    def frac_int(dv, tag):
                        # v = dv + 2 in (0.2, 3.8); e2 = floor(v); fr = v - e2
                        tt = fpool.tile([W, NK9], F32, tag=f"T{tag}")
                        t3 = fpool.tile([W, NK9], F32, tag=f"t3{tag}")
                        fr = fpool.tile([W, NK9], F32, tag=f"f{tag}")
                        e2 = fpool.tile([W, NK9], F32, tag=f"e{tag}")
                        nc.vector.tensor_scalar(out=pk2(tt), in0=dv, scalar1=2.0,
                                                scalar2=None, op0=AL.add)
                        nc.vector.tensor_scalar(out=t3[:], in0=tt[:],
                                                scalar1=-0.5, scalar2=MAGIC,
                                                op0=AL.add, op1=AL.add)
                        nc.vector.tensor_scalar(out=e2[:], in0=t3[:],
                                                scalar1=-MAGIC, scalar2=None,
                                                op0=AL.add)
                        nc.vector.tensor_sub(out=fr[:], in0=tt[:], in1=e2[:])
                        return fr, e2  # frac, floor+2 (exact int-valued)

                    fy, ey2 = frac_int(dyv, "y")
                    fx, ex2 = frac_int(dxv, "x")

                    def eq_pair(e2, lo, tag):
                        c0 = fpool.tile([W, NK9], F32, tag=f"c0{tag}")
                        c1 = fpool.tile([W, NK9], F32, tag=f"c1{tag}")
                        nc.vector.tensor_scalar(out=c0[:], in0=e2[:],
                                                scalar1=float(-(2 + lo)),
                                                scalar2=None, op0=AL.add)
                        nc.vector.tensor_scalar(out=c1[:], in0=c0[:], scalar1=1.0,
                                                scalar2=None, op0=AL.add)
                        eq0 = fpool.tile([W, NK9 * nty], F32, tag=f"eq0{tag}")
                        eq1 = fpool.tile([W, NK9 * nty], F32, tag=f"eq1{tag}")
                        itv = sb_view(ity_sb, 0,
                                      [[nty, W], [0, RB], [0, NTAP], [1, nty]])
                        for eq, cc in ((eq0, c0), (eq1, c1)):
                            nc.vector.tensor_tensor(
                                out=sb_view(eq, 0, [[NK9 * nty, W],
                                                    [NTAP * nty, RB],
                                                    [nty, NTAP], [1, nty]]),
                                in0=itv,
                                in1=sb_view(cc, 0, [[NK9, W], [NTAP, RB],
                                                    [1, NTAP], [0, nty]]),
                                op=AL.is_equal)
                        return eq0, eq1

                    eq0y, eq1y = eq_pair(ey2, TYLO, "y")
                    eq0x, eq1x = eq_pair(ex2, TXLO, "x")

                    def lerp(eq0, eq1, w1, w0, tag, n):
                        # -> eq0*w0 + eq1*w1  ([W, NK9*n])
                        res = fpool.tile([W, NK9 * n], F32, tag=f"lp{tag}")
                        et = fpool.tile([W, NK9 * n], F32, tag="lerptmp")
                        bc = lambda t: sb_view(t, 0, [[NK9, W], [NTAP, RB],
                                                      [1, NTAP], [0, n]])
                        fl = lambda t: sb_view(t, 0, [[NK9 * n, W],
                                                      [NTAP * n, RB],
                                                      [n, NTAP], [1, n]])
                        nc.vector.tensor_tensor(out=fl(res), in0=fl(eq0),
                                                in1=bc(w0), op=AL.mult)
                        nc.vector.tensor_tensor(out=fl(et), in0=fl(eq1),
                                                in1=bc(w1), op=AL.mult)
                        nc.vector.tensor_add(out=res[:], in0=res[:], in1=et[:])
                        return res

                    fy1 = fpool.tile([W, NK9], F32, tag="fy1")
                    nc.vector.tensor_scalar(out=fy1[:], in0=fy[:], scalar1=-1.0,
                                            scalar2=1.0, op0=AL.mult, op1=AL.add)
                    vy = lerp(eq0y, eq1y, fy, fy1, "vy", nty)
                    fxm = fpool.tile([W, NK9], F32, tag="fxm")
                    fx1m = fpool.tile([W, NK9], F32, tag="fx1m")
                    nc.vector.tensor_mul(out=fxm[:], in0=fx[:], in1=msk[:])
                    nc.vector.tensor_sub(out=fx1m[:], in0=msk[:], in1=fxm[:])
                    hxm = lerp(eq0x, eq1x, fxm, fx1m, "hx", ntx)

                    # shifted copies via DMA (partition-window exempt)
                    for u in US:
                        if u == 0:
                            continue
                        cnt = W - abs(u)
                        dlo, slo = max(0, u), max(0, -u)
                        nc.sync.dma_start(vy_u[u][dlo:dlo + cnt, :],
                                          vy[slo:slo + cnt, :])
                        nc.sync.dma_start(hxm_u[u][dlo:dlo + cnt, :],
                                          hxm[slo:slo + cnt, :])

                    # q_u[s, (hh,i), j, ty] = vy_u * hxm_u(tx=u-(j-1))
                    q_u = {}
                    for u in US:
                        jlo = max(0, u + 1 - TXHI)
                        jhi = min(2, u + 1 - TXLO)
                        nj = jhi - jlo + 1
                        vyt = vy_u[u] if u else vy
                        hxt = hxm_u[u] if u else hxm
                        qt = qpool.tile([W, RB * 3 * nj * nty], F32,
                                        tag=f"q{u}", name=f"q{u}")
                        nc.vector.tensor_tensor(
                            out=sb_view(qt, 0, [[RB * 3 * nj * nty, W],
                                                [nj * nty, 3 * RB],
                                                [nty, nj], [1, nty]]),
                            in0=bass.AP(
                                tensor=vyt.tensor,
                                offset=int(vyt.offset) + jlo * nty,
                                ap=[[NK9 * nty, W], [3 * nty, 3 * RB],
                                    [nty, nj], [1, nty]]),
                            in1=bass.AP(
                                tensor=hxt.tensor,
                                offset=int(hxt.offset) + jlo * ntx
                                + (u - (jlo - 1) - TXLO),
                                ap=[[NK9 * ntx, W], [3 * ntx, 3 * RB],
                                    [ntx - 1, nj], [0, nty]]),
                            op=AL.mult)
                        q_u[u] = (qt, jlo, nj)

                    # ---- stage 1: per-tap 1x1 convs into YT band ----------
                    yt = ytpool.tile([W, NROW * RS], F32, tag="yt")
                    for rr in range(NROW):
                        r = b0 - 3 + rr
                        if r < 0 or r >= H:
                            nc.vector.memset(yt[:, rr * RS:(rr + 1) * RS], 0.0)
                            continue
                        for g in range(3):
                            ps_y = ypool.tile([W, 3 * O], F32, tag="y")
                            lhsT = sb_view(xs, (r + 1) * Wp + 1,
                                           [[Hp * Wp, C], [1, W]])
                            nc.tensor.matmul(
                                ps_y[:], lhsT,
                                wmain_sb[:, g * 3 * O:(g + 1) * 3 * O],
                                start=True, stop=True)
                            nc.scalar.copy(
                                out=yt[:, rr * RS + g * 3 * O:
                                       rr * RS + (g + 1) * 3 * O],
                                in_=ps_y[:])

                    # ---- combine ------------------------------------------
                    obuf = opool.tile([O, RB * W], F32, tag="obuf")
                    for hh in range(RB):
                        ps_t = tpool.tile([O, W], F32, tag="pt")
                        for iu, u in enumerate(US):
                            qt, jlo, nj = q_u[u]
                            yt_off = (hh + 2 + TYLO) * RS + jlo * O
                            # balance multiplies: GpSimd ~60%, DVE keeps reduces
                            on_gps = u not in (0, -1)
                            eng = nc.gpsimd if on_gps else nc.vector
                            tmp = tmppool.tile(
                                [W, 3 * 3 * nty * O], F32,
                                tag=("tmpg" if on_gps else "tmp"), name="tmp")
                            for i in range(3):
                                # in0: YT[s, row = hh+3+(i-1)+ty, k=(i,j), o]
                                in0 = bass.AP(
                                    tensor=yt.tensor,
                                    offset=int(yt.offset) + yt_off
                                    + i * (RS + 3 * O),
                                    ap=[[NROW * RS, W], [O, nj],
                                        [RS, nty], [1, O]])
                                in1 = bass.AP(
                                    tensor=qt.tensor,
                                    offset=int(qt.offset)
                                    + hh * (3 * nj * nty) + i * (nj * nty),
                                    ap=[[RB * 3 * nj * nty, W],
                                        [nty, nj], [1, nty], [0, O]])
                                tfree = 3 * 3 * nty * O
                                # tmp memory [o][i][j][ty]: reduce walk contiguous
                                tout = sb_view(tmp, i * (nj * nty),
                                               [[tfree, W],
                                                [nty, nj],
                                                [1, nty], [3 * nj * nty, O]])
                                eng.tensor_tensor(out=tout, in0=in0,
                                                  in1=in1, op=AL.mult)
                            tred = sb_view(tmp, 0,
                                           [[tfree, W], [3 * nj * nty, O],
                                            [1, 3 * nj * nty]])
                            red = spool.tile([W, O], F32, tag="red")
                            nc.vector.tensor_reduce(
                                out=red[:], in_=tred,
                                axis=mybir.AxisListType.X, op=AL.add)
                            # psum[o, w] += red[s, o] with w = s - u
                            rhs = sb_view(identw_sb, 3 + u,
                                          [[W + 6, W], [1, W]])
                            nc.tensor.matmul(ps_t[:], red[:, :O], rhs,
                                             start=(iu == 0),
                                             stop=(iu == len(US) - 1))
                        nc.scalar.activation(out=obuf[:, hh * W:(hh + 1) * W],
                                             in_=ps_t[:], func=AF.Identity,
                                             bias=biaso_sb[:], scale=1.0)

                    nc.sync.dma_start(
                        bass.AP(tensor=out,
                                offset=img * O * H * W + b0 * W,
                                ap=[[H * W, O], [1, RB * W]]),
                        obuf[:])

    nc.compile()
    return nc


# ---------------------------------------------------------------------------
def _prep_host_inputs(x, weight, bias, offset_w, offset_b, mask_w, mask_b,
                      H, BS, nty):
    """Build per-core input maps (host-side layout marshalling only)."""
    B = x.shape[0]
    Hp, Wp = H + 2, W + 2
    ncores = B // BS
    xp = np.zeros((B, C, Hp, Wp), np.float32)
    xp[:, :, 1:1 + H, 1:1 + W] = x
    xp = xp.reshape(B, C, Hp * Wp)

    wmain = np.ascontiguousarray(
        weight.transpose(1, 2, 3, 0).reshape(C, NTAP * O)).astype(np.float32)
    wo = offset_w.transpose(1, 2, 3, 0)   # [C, 3, 3, 18]
    wm = mask_w.transpose(1, 2, 3, 0)     # [C, 3, 3, 9]
    womb = np.concatenate([wo, wm], axis=3).reshape(C, NTAP * 27)
    womb = np.ascontiguousarray(womb).astype(np.float32)
    ob27 = np.concatenate([offset_b, mask_b]).astype(np.float32)
    obrep = np.broadcast_to(ob27, (W, 27)).copy()
    ity = np.broadcast_to(np.arange(nty, dtype=np.float32), (W, nty)).copy()
    identw = np.zeros((W, W + 6), np.float32)
    identw[np.arange(W), np.arange(W) + 3] = 1.0
    biaso = bias.astype(np.float32).reshape(O, 1)

    shared = dict(wmain=wmain, womb=womb, obrep=obrep, ity=ity,
                  identw=identw, biaso=biaso)
    in_maps = []
    for corei in range(ncores):
        m = dict(shared)
        m["xp"] = np.ascontiguousarray(xp[corei * BS:(corei + 1) * BS])
        in_maps.append(m)
    return in_maps


_NC_CACHE = {}


def _get_nc(H=128, BS=2, RB=8, TYLO=-2, TYHI=2):
    key = (H, BS, RB, TYLO, TYHI)
    if key not in _NC_CACHE:
        _NC_CACHE[key] = build_nc(H, BS, RB, TYLO, TYHI)
    return _NC_CACHE[key]


def kernel(x, weight, bias, offset_w, offset_b, mask_w, mask_b):
    from concourse.bass_utils import run_bass_kernel_spmd

    x = np.asarray(x, np.float32)
    B, _, H, _ = x.shape
    BS = B // NCORES
    TYLO, TYHI = -2, 2
    nc = _get_nc(H=H, BS=BS)
    in_maps = _prep_host_inputs(
        x, np.asarray(weight), np.asarray(bias), np.asarray(offset_w),
        np.asarray(offset_b), np.asarray(mask_w), np.asarray(mask_b),
        H, BS, TYHI - TYLO + 1)
    res = run_bass_kernel_spmd(nc, in_maps, core_ids=list(range(NCORES)))
    outs = [res.results[i]["out"].reshape(BS, O, H, W) for i in range(NCORES)]
    return np.concatenate(outs, axis=0)


# revision 26
# speedup vs baseline: 1.0159x; 1.0159x over previous
"""DCNv2 (deformable conv) Trainium2 Bass kernel.

Strategy (per core, pure batch data-parallel across 8 cores):
  - x padded (+1) on host; SBUF resident per image: xs [96c, (H+2)(W+2)].
  - PE computes offset/mask 3x3 convs (9 accumulating matmuls per output row,
    pixels-on-partitions psum [128w, 27]) and per-tap 1x1 convs
    Y_k(o,r,w) = sum_c W[o,c,k] x(c,r,w) into a banded SBUF tensor
    YT [128w, (RB+6)rows, 9k, 96o] with zero row-halo.
  - DVE builds per-pixel bilinear row/col interpolation fields vy/hxm
    (integer sample-offset one-hots weighted by frac parts; mask folded
    into hxm).  Column shifts u are realized by DMA partition-shifted
    copies of vy/hxm (DMA is exempt from the partition-window rule).
  - DVE combine, per output row h and column shift u: one TT-mult with
    multi-dim APs over (i, j, ty, o) times q_u, then an XYZ tensor_reduce
    -> red_u[s, o] (s = source column).  PE merges the 7 shifted partials
    with shifted-identity matmuls accumulating psum[o, w]; ACT adds bias
    during the PSUM->SBUF copy; DMA out per band.
"""

import sys

sys.path.insert(0, "/opt/trn_rl_repo")

import numpy as np

import concourse.bacc as bacc
import concourse.bass as bass
import concourse.mybir as mybir
from concourse.tile import TileContext

F32 = mybir.dt.float32
AF = mybir.ActivationFunctionType
AL = mybir.AluOpType

C = 96
O = 96
NTAP = 9
W = 128
NCORES = 8


def build_nc(H=128, BS=2, RB=8, TYLO=-2, TYHI=2, num_devices=NCORES):
    """Build the per-core Bass program."""
    nty = TYHI - TYLO + 1
    ntx = nty
    TXLO, TXHI = TYLO, TYHI
    Hp, Wp = H + 2, W + 2
    RS = NTAP * O          # YT row stride = 864
    NROW = RB + 6          # YT band rows incl +-3 halo
    NK9 = RB * NTAP
    US = list(range(TXLO - 1, TXHI + 2))   # column shifts u = (j-1)+tx
    assert H % RB == 0

    nc = bacc.Bacc("TRN2", target_bir_lowering=False, debug=False,
                   num_devices=num_devices, dynamic_dma_scratch_size=2048)

    xp = nc.dram_tensor("xp", [BS, C, Hp * Wp], F32, kind="ExternalInput")
    wmain = nc.dram_tensor("wmain", [C, NTAP * O], F32, kind="ExternalInput")
    womb = nc.dram_tensor("womb", [C, NTAP * 27], F32, kind="ExternalInput")
    obrep = nc.dram_tensor("obrep", [W, 27], F32, kind="ExternalInput")
    ity = nc.dram_tensor("ity", [W, nty], F32, kind="ExternalInput")
    identw = nc.dram_tensor("identw", [W, W + 6], F32, kind="ExternalInput")
    biaso = nc.dram_tensor("biaso", [O, 1], F32, kind="ExternalInput")
    out = nc.dram_tensor("out", [BS, O, H * W], F32, kind="ExternalOutput")

    def sb_view(tile, offset, dims):
        return bass.AP(tensor=tile.tensor, offset=int(tile.offset) + offset,
                       ap=[list(d) for d in dims])

    with TileContext(nc) as tc:
        with (
            tc.tile_pool(name="consts", bufs=1) as cpool,
            tc.tile_pool(name="xs", bufs=1) as xpool,
            tc.tile_pool(name="yt", bufs=1) as ytpool,
            tc.tile_pool(name="fields", bufs=1) as fpool,
            tc.tile_pool(name="shf", bufs=1) as shpool,
            tc.tile_pool(name="qp", bufs=1) as qpool,
            tc.tile_pool(name="tmp", bufs=1) as tmppool,
            tc.tile_pool(name="small", bufs=4) as spool,
            tc.tile_pool(name="obuf", bufs=1) as opool,
            tc.tile_pool(name="psum_om", bufs=2, space="PSUM") as ompool,
            tc.tile_pool(name="psum_y", bufs=4, space="PSUM") as ypool,
            tc.tile_pool(name="psum_t", bufs=2, space="PSUM") as tpool,
        ):
            wmain_sb = cpool.tile([C, NTAP * O], F32)
            womb_sb = cpool.tile([C, NTAP * 27], F32)
            obrep_sb = cpool.tile([W, 27], F32)
            ity_sb = cpool.tile([W, nty], F32)
            identw_sb = cpool.tile([W, W + 6], F32)
            biaso_sb = cpool.tile([O, 1], F32)
            nc.sync.dma_start(wmain_sb[:], wmain[:])
            nc.sync.dma_start(womb_sb[:], womb[:])
            nc.sync.dma_start(obrep_sb[:], obrep[:])
            nc.sync.dma_start(ity_sb[:], ity[:])
            nc.sync.dma_start(identw_sb[:], identw[:])
            nc.sync.dma_start(biaso_sb[:], biaso[:])

            # partition-shifted field copies: value at partition s is the
            # field of destination column w = s - u; zero in the strips.
            vy_u = {}
            hxm_u = {}
            for u in US:
                if u == 0:
                    continue
                vy_u[u] = shpool.tile([W, NK9 * nty], F32, tag=f"vyu{u}", name=f"vyu{u}")
                hxm_u[u] = shpool.tile([W, NK9 * ntx], F32, tag=f"hxu{u}", name=f"hxu{u}")
                nc.vector.memset(vy_u[u][:], 0.0)
                nc.vector.memset(hxm_u[u][:], 0.0)

            for img in range(BS):
                xs = xpool.tile([C, Hp * Wp], F32, tag="xs")
                nc.sync.dma_start(
                    xs[:],
                    bass.AP(tensor=xp, offset=img * C * Hp * Wp,
                            ap=[[Hp * Wp, C], [1, Hp * Wp]]))

                for band in range(H // RB):
                    b0 = band * RB

                    # ---- offset/mask convs + raw fields -------------------
                    raw = fpool.tile([W, RB * 27], F32, tag="raw")
                    for hh in range(RB):
                        h = b0 + hh
                        ps_om = ompool.tile([W, 27], F32, tag="om")
                        for t in range(NTAP):
                            ti, tj = t // 3, t % 3
                            lhsT = sb_view(xs, (h + ti) * Wp + tj,
                                           [[Hp * Wp, C], [1, W]])
                            nc.tensor.matmul(ps_om[:], lhsT,
                                             womb_sb[:, t * 27:(t + 1) * 27],
                                             start=(t == 0), stop=(t == NTAP - 1))
                        nc.vector.tensor_add(
                            out=raw[:, hh * 27:(hh + 1) * 27],
                            in0=ps_om[:], in1=obrep_sb[:])

                    # ---- per-pixel interpolation fields -------------------
                    dyv = sb_view(raw, 0, [[RB * 27, W], [27, RB], [2, NTAP]])
                    dxv = sb_view(raw, 1, [[RB * 27, W], [27, RB], [2, NTAP]])
                    mrawv = sb_view(raw, 18, [[RB * 27, W], [27, RB], [1, NTAP]])

                    def pk2(tile):  # packed [hh][k] view matching (RB, NTAP)
                        return sb_view(tile, 0, [[NK9, W], [NTAP, RB], [1, NTAP]])

                    msk = fpool.tile([W, NK9], F32, tag="msk")
                    nc.scalar.activation(out=pk2(msk), in_=mrawv, func=AF.Sigmoid)

                    MAGIC = 12582912.0  # 1.5 * 2**23: fp32 round-to-int magic

                    def frac_int(dv, tag):
                        # v = dv + 2 in (0.2, 3.8); e2 = floor(v); fr = v - e2
                        tt = fpool.tile([W, NK9], F32, tag=f"T{tag}")
                        t3 = fpool.tile([W, NK9], F32, tag=f"t3{tag}")
                        fr = fpool.tile([W, NK9], F32, tag=f"f{tag}")
                        e2 = fpool.tile([W, NK9], F32, tag=f"e{tag}")
                        nc.vector.tensor_scalar(out=pk2(tt), in0=dv, scalar1=2.0,
                                                scalar2=None, op0=AL.add)
                        nc.vector.tensor_scalar(out=t3[:], in0=tt[:],
                                                scalar1=-0.5, scalar2=MAGIC,
                                                op0=AL.add, op1=AL.add)
                        nc.vector.tensor_scalar(out=e2[:], in0=t3[:],
                                                scalar1=-MAGIC, scalar2=None,
                                                op0=AL.add)
                        nc.vector.tensor_sub(out=fr[:], in0=tt[:], in1=e2[:])
                        return fr, e2  # frac, floor+2 (exact int-valued)

                    fy, ey2 = frac_int(dyv, "y")
                    fx, ex2 = frac_int(dxv, "x")

                    def eq_pair(e2, lo, tag):
                        c0 = fpool.tile([W, NK9], F32, tag=f"c0{tag}")
                        c1 = fpool.tile([W, NK9], F32, tag=f"c1{tag}")
                        nc.vector.tensor_scalar(out=c0[:], in0=e2[:],
                                                scalar1=float(-(2 + lo)),
                                                scalar2=None, op0=AL.add)
                        nc.vector.tensor_scalar(out=c1[:], in0=c0[:], scalar1=1.0,
                                                scalar2=None, op0=AL.add)
                        eq0 = fpool.tile([W, NK9 * nty], F32, tag=f"eq0{tag}")
                        eq1 = fpool.tile([W, NK9 * nty], F32, tag=f"eq1{tag}")
                        itv = sb_view(ity_sb, 0,
                                      [[nty, W], [0, RB], [0, NTAP], [1, nty]])
                        for eq, cc in ((eq0, c0), (eq1, c1)):
                            nc.vector.tensor_tensor(
                                out=sb_view(eq, 0, [[NK9 * nty, W],
                                                    [NTAP * nty, RB],
                                                    [nty, NTAP], [1, nty]]),
                                in0=itv,
                                in1=sb_view(cc, 0, [[NK9, W], [NTAP, RB],
                                                    [1, NTAP], [0, nty]]),
                                op=AL.is_equal)
                        return eq0, eq1

                    eq0y, eq1y = eq_pair(ey2, TYLO, "y")
                    eq0x, eq1x = eq_pair(ex2, TXLO, "x")

                    def lerp(eq0, eq1, w1, w0, tag, n):
                        # -> eq0*w0 + eq1*w1  ([W, NK9*n])
                        res = fpool.tile([W, NK9 * n], F32, tag=f"lp{tag}")
                        et = fpool.tile([W, NK9 * n], F32, tag="lerptmp")
                        bc = lambda t: sb_view(t, 0, [[NK9, W], [NTAP, RB],
                                                      [1, NTAP], [0, n]])
                        fl = lambda t: sb_view(t, 0, [[NK9 * n, W],
                                                      [NTAP * n, RB],
                                                      [n, NTAP], [1, n]])
                        nc.vector.tensor_tensor(out=fl(res), in0=fl(eq0),
                                                in1=bc(w0), op=AL.mult)
                        nc.vector.tensor_tensor(out=fl(et), in0=fl(eq1),
                                                in1=bc(w1), op=AL.mult)
                        nc.vector.tensor_add(out=res[:], in0=res[:], in1=et[:])
                        return res

                    fy1 = fpool.tile([W, NK9], F32, tag="fy1")
                    nc.vector.tensor_scalar(out=fy1[:], in0=fy[:], scalar1=-1.0,
                                            scalar2=1.0, op0=AL.mult, op1=AL.add)
                    vy = lerp(eq0y, eq1y, fy, fy1, "vy", nty)
                    fxm = fpool.tile([W, NK9], F32, tag="fxm")
                    fx1m = fpool.tile([W, NK9], F32, tag="fx1m")
                    nc.vector.tensor_mul(out=fxm[:], in0=fx[:], in1=msk[:])
                    nc.vector.tensor_sub(out=fx1m[:], in0=msk[:], in1=fxm[:])
                    hxm = lerp(eq0x, eq1x, fxm, fx1m, "hx", ntx)

                    # shifted copies via DMA (partition-window exempt)
                    for u in US:
                        if u == 0:
                            continue
                        cnt = W - abs(u)
                        dlo, slo = max(0, u), max(0, -u)
                        nc.sync.dma_start(vy_u[u][dlo:dlo + cnt, :],
                                          vy[slo:slo + cnt, :])
                        nc.sync.dma_start(hxm_u[u][dlo:dlo + cnt, :],
                                          hxm[slo:slo + cnt, :])

                    # q_u[s, (hh,i), j, ty] = vy_u * hxm_u(tx=u-(j-1))
                    q_u = {}
                    for u in US:
                        jlo = max(0, u + 1 - TXHI)
                        jhi = min(2, u + 1 - TXLO)
                        nj = jhi - jlo + 1
                        vyt = vy_u[u] if u else vy
                        hxt = hxm_u[u] if u else hxm
                        qt = qpool.tile([W, RB * 3 * nj * nty], F32,
                                        tag=f"q{u}", name=f"q{u}")
                        nc.vector.tensor_tensor(
                            out=sb_view(qt, 0, [[RB * 3 * nj * nty, W],
                                                [nj * nty, 3 * RB],
                                                [nty, nj], [1, nty]]),
                            in0=bass.AP(
                                tensor=vyt.tensor,
                                offset=int(vyt.offset) + jlo * nty,
                                ap=[[NK9 * nty, W], [3 * nty, 3 * RB],
                                    [nty, nj], [1, nty]]),
                            in1=bass.AP(
                                tensor=hxt.tensor,
                                offset=int(hxt.offset) + jlo * ntx
                                + (u - (jlo - 1) - TXLO),
                                ap=[[NK9 * ntx, W], [3 * ntx, 3 * RB],
                                    [ntx - 1, nj], [0, nty]]),
                            op=AL.mult)
                        q_u[u] = (qt, jlo, nj)

                    # ---- stage 1: per-tap 1x1 convs into YT band ----------
                    yt = ytpool.tile([W, NROW * RS], F32, tag="yt")
                    for rr in range(NROW):
                        r = b0 - 3 + rr
                        if r < 0 or r >= H:
                            nc.vector.memset(yt[:, rr * RS:(rr + 1) * RS], 0.0)
                            continue
                        for g in range(3):
                            ps_y = ypool.tile([W, 3 * O], F32, tag="y")
                            lhsT = sb_view(xs, (r + 1) * Wp + 1,
                                           [[Hp * Wp, C], [1, W]])
                            nc.tensor.matmul(
                                ps_y[:], lhsT,
                                wmain_sb[:, g * 3 * O:(g + 1) * 3 * O],
                                start=True, stop=True)
                            nc.scalar.copy(
                                out=yt[:, rr * RS + g * 3 * O:
                                       rr * RS + (g + 1) * 3 * O],
                                in_=ps_y[:])

                    # ---- combine ------------------------------------------
                    obuf = opool.tile([O, RB * W], F32, tag="obuf")
                    UORD = [0, -2, -1, 2, 1, -3, 3] if len(US) == 7 else US
                    for hh in range(RB):
                        ps_t = tpool.tile([O, W], F32, tag="pt")
                        for iu, u in enumerate(UORD):
                            qt, jlo, nj = q_u[u]
                            yt_off = (hh + 2 + TYLO) * RS + jlo * O
                            # balance multiplies: GpSimd ~60%, DVE keeps reduces
                            on_gps = abs(u) >= 2
                            eng = nc.gpsimd if on_gps else nc.vector
                            tmp = tmppool.tile(
                                [W, 3 * 3 * nty * O], F32,
                                tag=("tmpg" if on_gps else "tmp"), name="tmp")
                            for i in range(3):
                                # in0: YT[s, row = hh+3+(i-1)+ty, k=(i,j), o]
                                in0 = bass.AP(
                                    tensor=yt.tensor,
                                    offset=int(yt.offset) + yt_off
                                    + i * (RS + 3 * O),
                                    ap=[[NROW * RS, W], [O, nj],
                                        [RS, nty], [1, O]])
                                in1 = bass.AP(
                                    tensor=qt.tensor,
                                    offset=int(qt.offset)
                                    + hh * (3 * nj * nty) + i * (nj * nty),
                                    ap=[[RB * 3 * nj * nty, W],
                                        [nty, nj], [1, nty], [0, O]])
                                tfree = 3 * 3 * nty * O
                                # tmp memory [o][i][j][ty]: reduce walk contiguous
                                tout = sb_view(tmp, i * (nj * nty),
                                               [[tfree, W],
                                                [nty, nj],
                                                [1, nty], [3 * nj * nty, O]])
                                eng.tensor_tensor(out=tout, in0=in0,
                                                  in1=in1, op=AL.mult)
                            tred = sb_view(tmp, 0,
                                           [[tfree, W], [3 * nj * nty, O],
                                            [1, 3 * nj * nty]])
                            red = spool.tile([W, O], F32, tag="red")
                            nc.vector.tensor_reduce(
                                out=red[:], in_=tred,
                                axis=mybir.AxisListType.X, op=AL.add)
                            # psum[o, w] += red[s, o] with w = s - u
                            rhs = sb_view(identw_sb, 3 + u,
                                          [[W + 6, W], [1, W]])
                            nc.tensor.matmul(ps_t[:], red[:, :O], rhs,
                                             start=(iu == 0),
                                             stop=(iu == len(UORD) - 1))
                        nc.scalar.activation(out=obuf[:, hh * W:(hh + 1) * W],
                                             in_=ps_t[:], func=AF.Identity,
                                             bias=biaso_sb[:], scale=1.0)

                    nc.sync.dma_start(
                        bass.AP(tensor=out,
                                offset=img * O * H * W + b0 * W,
                                ap=[[H * W, O], [1, RB * W]]),
                        obuf[:])

    nc.compile()
    return nc


# ---------------------------------------------------------------------------
def _prep_host_inputs(x, weight, bias, offset_w, offset_b, mask_w, mask_b,
                      H, BS, nty):
    """Build per-core input maps (host-side layout marshalling only)."""
    B = x.shape[0]
    Hp, Wp = H + 2, W + 2
    ncores = B // BS
    xp = np.zeros((B, C, Hp, Wp), np.float32)
    xp[:, :, 1:1 + H, 1:1 + W] = x
    xp = xp.reshape(B, C, Hp * Wp)

    wmain = np.ascontiguousarray(
        weight.transpose(1, 2, 3, 0).reshape(C, NTAP * O)).astype(np.float32)
    wo = offset_w.transpose(1, 2, 3, 0)   # [C, 3, 3, 18]
    wm = mask_w.transpose(1, 2, 3, 0)     # [C, 3, 3, 9]
    womb = np.concatenate([wo, wm], axis=3).reshape(C, NTAP * 27)
    womb = np.ascontiguousarray(womb).astype(np.float32)
    ob27 = np.concatenate([offset_b, mask_b]).astype(np.float32)
    obrep = np.broadcast_to(ob27, (W, 27)).copy()
    ity = np.broadcast_to(np.arange(nty, dtype=np.float32), (W, nty)).copy()
    identw = np.zeros((W, W + 6), np.float32)
    identw[np.arange(W), np.arange(W) + 3] = 1.0
    biaso = bias.astype(np.float32).reshape(O, 1)

    shared = dict(wmain=wmain, womb=womb, obrep=obrep, ity=ity,
                  identw=identw, biaso=biaso)
    in_maps = []
    for corei in range(ncores):
        m = dict(shared)
        m["xp"] = np.ascontiguousarray(xp[corei * BS:(corei + 1) * BS])
        in_maps.append(m)
    return in_maps


_NC_CACHE = {}


def _get_nc(H=128, BS=2, RB=8, TYLO=-2, TYHI=2):
    key = (H, BS, RB, TYLO, TYHI)
    if key not in _NC_CACHE:
        _NC_CACHE[key] = build_nc(H, BS, RB, TYLO, TYHI)
    return _NC_CACHE[key]


def kernel(x, weight, bias, offset_w, offset_b, mask_w, mask_b):
    from concourse.bass_utils import run_bass_kernel_spmd

    x = np.asarray(x, np.float32)
    B, _, H, _ = x.shape
    BS = B // NCORES
    TYLO, TYHI = -2, 2
    nc = _get_nc(H=H, BS=BS)
    in_maps = _prep_host_inputs(
        x, np.asarray(weight), np.asarray(bias), np.asarray(offset_w),
        np.asarray(offset_b), np.asarray(mask_w), np.asarray(mask_b),
        H, BS, TYHI - TYLO + 1)
    res = run_bass_kernel_spmd(nc, in_maps, core_ids=list(range(NCORES)))
    outs = [res.results[i]["out"].reshape(BS, O, H, W) for i in range(NCORES)]
    return np.concatenate(outs, axis=0)


# revision 27
# speedup vs baseline: 1.0314x; 1.0153x over previous
"""DCNv2 (deformable conv) Trainium2 Bass kernel.

Strategy (per core, pure batch data-parallel across 8 cores):
  - x padded (+1) on host; SBUF resident per image: xs [96c, (H+2)(W+2)].
  - PE computes offset/mask 3x3 convs (9 accumulating matmuls per output row,
    pixels-on-partitions psum [128w, 27]) and per-tap 1x1 convs
    Y_k(o,r,w) = sum_c W[o,c,k] x(c,r,w) into a banded SBUF tensor
    YT [128w, (RB+6)rows, 9k, 96o] with zero row-halo.
  - DVE builds per-pixel bilinear row/col interpolation fields vy/hxm
    (integer sample-offset one-hots weighted by frac parts; mask folded
    into hxm).  Column shifts u are realized by DMA partition-shifted
    copies of vy/hxm (DMA is exempt from the partition-window rule).
  - DVE combine, per output row h and column shift u: one TT-mult with
    multi-dim APs over (i, j, ty, o) times q_u, then an XYZ tensor_reduce
    -> red_u[s, o] (s = source column).  PE merges the 7 shifted partials
    with shifted-identity matmuls accumulating psum[o, w]; ACT adds bias
    during the PSUM->SBUF copy; DMA out per band.
"""

import sys

sys.path.insert(0, "/opt/trn_rl_repo")

import numpy as np

import concourse.bacc as bacc
import concourse.bass as bass
import concourse.mybir as mybir
from concourse.tile import TileContext

F32 = mybir.dt.float32
AF = mybir.ActivationFunctionType
AL = mybir.AluOpType

C = 96
O = 96
NTAP = 9
W = 128
NCORES = 8


def build_nc(H=128, BS=2, RB=8, TYLO=-2, TYHI=2, num_devices=NCORES):
    """Build the per-core Bass program."""
    nty = TYHI - TYLO + 1
    ntx = nty
    TXLO, TXHI = TYLO, TYHI
    Hp, Wp = H + 2, W + 2
    RS = NTAP * O          # YT row stride = 864
    NROW = RB + 6          # YT band rows incl +-3 halo
    NK9 = RB * NTAP
    US = list(range(TXLO - 1, TXHI + 2))   # column shifts u = (j-1)+tx
    assert H % RB == 0

    nc = bacc.Bacc("TRN2", target_bir_lowering=False, debug=False,
                   num_devices=num_devices, dynamic_dma_scratch_size=2048)

    xp = nc.dram_tensor("xp", [BS, C, Hp * Wp], F32, kind="ExternalInput")
    wmain = nc.dram_tensor("wmain", [C, NTAP * O], F32, kind="ExternalInput")
    womb = nc.dram_tensor("womb", [C, NTAP * 27], F32, kind="ExternalInput")
    obrep = nc.dram_tensor("obrep", [W, 27], F32, kind="ExternalInput")
    ity = nc.dram_tensor("ity", [W, nty], F32, kind="ExternalInput")
    identw = nc.dram_tensor("identw", [W, W + 6], F32, kind="ExternalInput")
    biaso = nc.dram_tensor("biaso", [O, 1], F32, kind="ExternalInput")
    out = nc.dram_tensor("out", [BS, O, H * W], F32, kind="ExternalOutput")

    def sb_view(tile, offset, dims):
        return bass.AP(tensor=tile.tensor, offset=int(tile.offset) + offset,
                       ap=[list(d) for d in dims])

    with TileContext(nc) as tc:
        with (
            tc.tile_pool(name="consts", bufs=1) as cpool,
            tc.tile_pool(name="xs", bufs=2) as xpool,
            tc.tile_pool(name="yt", bufs=1) as ytpool,
            tc.tile_pool(name="fields", bufs=1) as fpool,
            tc.tile_pool(name="shf", bufs=1) as shpool,
            tc.tile_pool(name="qp", bufs=1) as qpool,
            tc.tile_pool(name="tmp", bufs=2) as tmppool,
            tc.tile_pool(name="small", bufs=4) as spool,
            tc.tile_pool(name="obuf", bufs=1) as opool,
            tc.tile_pool(name="psum_om", bufs=2, space="PSUM") as ompool,
            tc.tile_pool(name="psum_y", bufs=4, space="PSUM") as ypool,
            tc.tile_pool(name="psum_t", bufs=2, space="PSUM") as tpool,
        ):
            wmain_sb = cpool.tile([C, NTAP * O], F32)
            womb_sb = cpool.tile([C, NTAP * 27], F32)
            obrep_sb = cpool.tile([W, 27], F32)
            ity_sb = cpool.tile([W, nty], F32)
            identw_sb = cpool.tile([W, W + 6], F32)
            biaso_sb = cpool.tile([O, 1], F32)
            nc.sync.dma_start(wmain_sb[:], wmain[:])
            nc.sync.dma_start(womb_sb[:], womb[:])
            nc.sync.dma_start(obrep_sb[:], obrep[:])
            nc.sync.dma_start(ity_sb[:], ity[:])
            nc.sync.dma_start(identw_sb[:], identw[:])
            nc.sync.dma_start(biaso_sb[:], biaso[:])

            # partition-shifted field copies: value at partition s is the
            # field of destination column w = s - u; zero in the strips.
            vy_u = {}
            hxm_u = {}
            for u in US:
                if u == 0:
                    continue
                vy_u[u] = shpool.tile([W, NK9 * nty], F32, tag=f"vyu{u}", name=f"vyu{u}")
                hxm_u[u] = shpool.tile([W, NK9 * ntx], F32, tag=f"hxu{u}", name=f"hxu{u}")
                nc.vector.memset(vy_u[u][:], 0.0)
                nc.vector.memset(hxm_u[u][:], 0.0)

            XBROW = RB + 6  # band x rows: padded rows [b0-2, b0+RB+4)
            for img in range(BS):
                for band in range(H // RB):
                    b0 = band * RB
                    xs = xpool.tile([C, XBROW * Wp], F32, tag="xs")
                    rlo = max(0, b0 - 2)
                    rhi = min(Hp, b0 + RB + 4)
                    dst0 = (rlo - (b0 - 2)) * Wp
                    nc.sync.dma_start(
                        xs[:, dst0:dst0 + (rhi - rlo) * Wp],
                        bass.AP(tensor=xp,
                                offset=img * C * Hp * Wp + rlo * Wp,
                                ap=[[Hp * Wp, C], [1, (rhi - rlo) * Wp]]))

                    # ---- offset/mask convs + raw fields -------------------
                    raw = fpool.tile([W, RB * 27], F32, tag="raw")
                    for hh in range(RB):
                        h = b0 + hh
                        ps_om = ompool.tile([W, 27], F32, tag="om")
                        for t in range(NTAP):
                            ti, tj = t // 3, t % 3
                            lhsT = sb_view(xs, (hh + ti + 2) * Wp + tj,
                                           [[XBROW * Wp, C], [1, W]])
                            nc.tensor.matmul(ps_om[:], lhsT,
                                             womb_sb[:, t * 27:(t + 1) * 27],
                                             start=(t == 0), stop=(t == NTAP - 1))
                        nc.vector.tensor_add(
                            out=raw[:, hh * 27:(hh + 1) * 27],
                            in0=ps_om[:], in1=obrep_sb[:])

                    # ---- per-pixel interpolation fields -------------------
                    dyv = sb_view(raw, 0, [[RB * 27, W], [27, RB], [2, NTAP]])
                    dxv = sb_view(raw, 1, [[RB * 27, W], [27, RB], [2, NTAP]])
                    mrawv = sb_view(raw, 18, [[RB * 27, W], [27, RB], [1, NTAP]])

                    def pk2(tile):  # packed [hh][k] view matching (RB, NTAP)
                        return sb_view(tile, 0, [[NK9, W], [NTAP, RB], [1, NTAP]])

                    msk = fpool.tile([W, NK9], F32, tag="msk")
                    nc.scalar.activation(out=pk2(msk), in_=mrawv, func=AF.Sigmoid)

                    MAGIC = 12582912.0  # 1.5 * 2**23: fp32 round-to-int magic

                    def frac_int(dv, tag):
                        # v = dv + 2 in (0.2, 3.8); e2 = floor(v); fr = v - e2
                        tt = fpool.tile([W, NK9], F32, tag=f"T{tag}")
                        t3 = fpool.tile([W, NK9], F32, tag=f"t3{tag}")
                        fr = fpool.tile([W, NK9], F32, tag=f"f{tag}")
                        e2 = fpool.tile([W, NK9], F32, tag=f"e{tag}")
                        nc.vector.tensor_scalar(out=pk2(tt), in0=dv, scalar1=2.0,
                                                scalar2=None, op0=AL.add)
                        nc.vector.tensor_scalar(out=t3[:], in0=tt[:],
                                                scalar1=-0.5, scalar2=MAGIC,
                                                op0=AL.add, op1=AL.add)
                        nc.vector.tensor_scalar(out=e2[:], in0=t3[:],
                                                scalar1=-MAGIC, scalar2=None,
                                                op0=AL.add)
                        nc.vector.tensor_sub(out=fr[:], in0=tt[:], in1=e2[:])
                        return fr, e2  # frac, floor+2 (exact int-valued)

                    fy, ey2 = frac_int(dyv, "y")
                    fx, ex2 = frac_int(dxv, "x")

                    def eq_pair(e2, lo, tag):
                        c0 = fpool.tile([W, NK9], F32, tag=f"c0{tag}")
                        c1 = fpool.tile([W, NK9], F32, tag=f"c1{tag}")
                        nc.vector.tensor_scalar(out=c0[:], in0=e2[:],
                                                scalar1=float(-(2 + lo)),
                                                scalar2=None, op0=AL.add)
                        nc.vector.tensor_scalar(out=c1[:], in0=c0[:], scalar1=1.0,
                                                scalar2=None, op0=AL.add)
                        eq0 = fpool.tile([W, NK9 * nty], F32, tag=f"eq0{tag}")
                        eq1 = fpool.tile([W, NK9 * nty], F32, tag=f"eq1{tag}")
                        itv = sb_view(ity_sb, 0,
                                      [[nty, W], [0, RB], [0, NTAP], [1, nty]])
                        for eq, cc in ((eq0, c0), (eq1, c1)):
                            nc.vector.tensor_tensor(
                                out=sb_view(eq, 0, [[NK9 * nty, W],
                                                    [NTAP * nty, RB],
                                                    [nty, NTAP], [1, nty]]),
                                in0=itv,
                                in1=sb_view(cc, 0, [[NK9, W], [NTAP, RB],
                                                    [1, NTAP], [0, nty]]),
                                op=AL.is_equal)
                        return eq0, eq1

                    eq0y, eq1y = eq_pair(ey2, TYLO, "y")
                    eq0x, eq1x = eq_pair(ex2, TXLO, "x")

                    def lerp(eq0, eq1, w1, w0, tag, n):
                        # -> eq0*w0 + eq1*w1  ([W, NK9*n])
                        res = fpool.tile([W, NK9 * n], F32, tag=f"lp{tag}")
                        et = fpool.tile([W, NK9 * n], F32, tag="lerptmp")
                        bc = lambda t: sb_view(t, 0, [[NK9, W], [NTAP, RB],
                                                      [1, NTAP], [0, n]])
                        fl = lambda t: sb_view(t, 0, [[NK9 * n, W],
                                                      [NTAP * n, RB],
                                                      [n, NTAP], [1, n]])
                        nc.vector.tensor_tensor(out=fl(res), in0=fl(eq0),
                                                in1=bc(w0), op=AL.mult)
                        nc.vector.tensor_tensor(out=fl(et), in0=fl(eq1),
                                                in1=bc(w1), op=AL.mult)
                        nc.vector.tensor_add(out=res[:], in0=res[:], in1=et[:])
                        return res

                    fy1 = fpool.tile([W, NK9], F32, tag="fy1")
                    nc.vector.tensor_scalar(out=fy1[:], in0=fy[:], scalar1=-1.0,
                                            scalar2=1.0, op0=AL.mult, op1=AL.add)
                    vy = lerp(eq0y, eq1y, fy, fy1, "vy", nty)
                    fxm = fpool.tile([W, NK9], F32, tag="fxm")
                    fx1m = fpool.tile([W, NK9], F32, tag="fx1m")
                    nc.vector.tensor_mul(out=fxm[:], in0=fx[:], in1=msk[:])
                    nc.vector.tensor_sub(out=fx1m[:], in0=msk[:], in1=fxm[:])
                    hxm = lerp(eq0x, eq1x, fxm, fx1m, "hx", ntx)

                    # shifted copies via DMA (partition-window exempt)
                    for u in US:
                        if u == 0:
                            continue
                        cnt = W - abs(u)
                        dlo, slo = max(0, u), max(0, -u)
                        nc.sync.dma_start(vy_u[u][dlo:dlo + cnt, :],
                                          vy[slo:slo + cnt, :])
                        nc.sync.dma_start(hxm_u[u][dlo:dlo + cnt, :],
                                          hxm[slo:slo + cnt, :])

                    # q_u[s, (hh,i), j, ty] = vy_u * hxm_u(tx=u-(j-1))
                    q_u = {}
                    for u in US:
                        jlo = max(0, u + 1 - TXHI)
                        jhi = min(2, u + 1 - TXLO)
                        nj = jhi - jlo + 1
                        vyt = vy_u[u] if u else vy
                        hxt = hxm_u[u] if u else hxm
                        qt = qpool.tile([W, RB * 3 * nj * nty], F32,
                                        tag=f"q{u}", name=f"q{u}")
                        nc.vector.tensor_tensor(
                            out=sb_view(qt, 0, [[RB * 3 * nj * nty, W],
                                                [nj * nty, 3 * RB],
                                                [nty, nj], [1, nty]]),
                            in0=bass.AP(
                                tensor=vyt.tensor,
                                offset=int(vyt.offset) + jlo * nty,
                                ap=[[NK9 * nty, W], [3 * nty, 3 * RB],
                                    [nty, nj], [1, nty]]),
                            in1=bass.AP(
                                tensor=hxt.tensor,
                                offset=int(hxt.offset) + jlo * ntx
                                + (u - (jlo - 1) - TXLO),
                                ap=[[NK9 * ntx, W], [3 * ntx, 3 * RB],
                                    [ntx - 1, nj], [0, nty]]),
                            op=AL.mult)
                        q_u[u] = (qt, jlo, nj)

                    # ---- stage 1: per-tap 1x1 convs into YT band ----------
                    yt = ytpool.tile([W, NROW * RS], F32, tag="yt")
                    for rr in range(NROW):
                        r = b0 - 3 + rr
                        if r < 0 or r >= H:
                            nc.vector.memset(yt[:, rr * RS:(rr + 1) * RS], 0.0)
                            continue
                        for g in range(3):
                            ps_y = ypool.tile([W, 3 * O], F32, tag="y")
                            lhsT = sb_view(xs, rr * Wp + 1,
                                           [[XBROW * Wp, C], [1, W]])
                            nc.tensor.matmul(
                                ps_y[:], lhsT,
                                wmain_sb[:, g * 3 * O:(g + 1) * 3 * O],
                                start=True, stop=True)
                            nc.scalar.copy(
                                out=yt[:, rr * RS + g * 3 * O:
                                       rr * RS + (g + 1) * 3 * O],
                                in_=ps_y[:])

                    # ---- combine ------------------------------------------
                    obuf = opool.tile([O, RB * W], F32, tag="obuf")
                    UORD = [0, -2, -1, 2, 1, -3, 3] if len(US) == 7 else US
                    for hh in range(RB):
                        ps_t = tpool.tile([O, W], F32, tag="pt")
                        for iu, u in enumerate(UORD):
                            qt, jlo, nj = q_u[u]
                            yt_off = (hh + 2 + TYLO) * RS + jlo * O
                            # balance multiplies: GpSimd ~60%, DVE keeps reduces
                            on_gps = abs(u) >= 2
                            eng = nc.gpsimd if on_gps else nc.vector
                            tmp = tmppool.tile(
                                [W, 3 * 3 * nty * O], F32,
                                tag=("tmpg" if on_gps else "tmp"), name="tmp")
                            for i in range(3):
                                # in0: YT[s, row = hh+3+(i-1)+ty, k=(i,j), o]
                                in0 = bass.AP(
                                    tensor=yt.tensor,
                                    offset=int(yt.offset) + yt_off
                                    + i * (RS + 3 * O),
                                    ap=[[NROW * RS, W], [O, nj],
                                        [RS, nty], [1, O]])
                                in1 = bass.AP(
                                    tensor=qt.tensor,
                                    offset=int(qt.offset)
                                    + hh * (3 * nj * nty) + i * (nj * nty),
                                    ap=[[RB * 3 * nj * nty, W],
                                        [nty, nj], [1, nty], [0, O]])
                                tfree = 3 * 3 * nty * O
                                # tmp memory [o][i][j][ty]: reduce walk contiguous
                                tout = sb_view(tmp, i * (nj * nty),
                                               [[tfree, W],
                                                [nty, nj],
                                                [1, nty], [3 * nj * nty, O]])
                                eng.tensor_tensor(out=tout, in0=in0,
                                                  in1=in1, op=AL.mult)
                            tred = sb_view(tmp, 0,
                                           [[tfree, W], [3 * nj * nty, O],
                                            [1, 3 * nj * nty]])
                            red = spool.tile([W, O], F32, tag="red")
                            nc.vector.tensor_reduce(
                                out=red[:], in_=tred,
                                axis=mybir.AxisListType.X, op=AL.add)
                            # psum[o, w] += red[s, o] with w = s - u
                            rhs = sb_view(identw_sb, 3 + u,
                                          [[W + 6, W], [1, W]])
                            nc.tensor.matmul(ps_t[:], red[:, :O], rhs,
                                             start=(iu == 0),
                                             stop=(iu == len(UORD) - 1))
                        nc.scalar.activation(out=obuf[:, hh * W:(hh + 1) * W],
                                             in_=ps_t[:], func=AF.Identity,
                                             bias=biaso_sb[:], scale=1.0)

                    nc.sync.dma_start(
                        bass.AP(tensor=out,
                                offset=img * O * H * W + b0 * W,
                                ap=[[H * W, O], [1, RB * W]]),
                        obuf[:])

    nc.compile()
    return nc


# ---------------------------------------------------------------------------
def _prep_host_inputs(x, weight, bias, offset_w, offset_b, mask_w, mask_b,
                      H, BS, nty):
    """Build per-core input maps (host-side layout marshalling only)."""
    B = x.shape[0]
    Hp, Wp = H + 2, W + 2
    ncores = B // BS
    xp = np.zeros((B, C, Hp, Wp), np.float32)
    xp[:, :, 1:1 + H, 1:1 + W] = x
    xp = xp.reshape(B, C, Hp * Wp)

    wmain = np.ascontiguousarray(
        weight.transpose(1, 2, 3, 0).reshape(C, NTAP * O)).astype(np.float32)
    wo = offset_w.transpose(1, 2, 3, 0)   # [C, 3, 3, 18]
    wm = mask_w.transpose(1, 2, 3, 0)     # [C, 3, 3, 9]
    womb = np.concatenate([wo, wm], axis=3).reshape(C, NTAP * 27)
    womb = np.ascontiguousarray(womb).astype(np.float32)
    ob27 = np.concatenate([offset_b, mask_b]).astype(np.float32)
    obrep = np.broadcast_to(ob27, (W, 27)).copy()
    ity = np.broadcast_to(np.arange(nty, dtype=np.float32), (W, nty)).copy()
    identw = np.zeros((W, W + 6), np.float32)
    identw[np.arange(W), np.arange(W) + 3] = 1.0
    biaso = bias.astype(np.float32).reshape(O, 1)

    shared = dict(wmain=wmain, womb=womb, obrep=obrep, ity=ity,
                  identw=identw, biaso=biaso)
    in_maps = []
    for corei in range(ncores):
        m = dict(shared)
        m["xp"] = np.ascontiguousarray(xp[corei * BS:(corei + 1) * BS])
        in_maps.append(m)
    return in_maps


_NC_CACHE = {}


def _get_nc(H=128, BS=2, RB=8, TYLO=-2, TYHI=2):
    key = (H, BS, RB, TYLO, TYHI)
    if key not in _NC_CACHE:
        _NC_CACHE[key] = build_nc(H, BS, RB, TYLO, TYHI)
    return _NC_CACHE[key]


def kernel(x, weight, bias, offset_w, offset_b, mask_w, mask_b):
    from concourse.bass_utils import run_bass_kernel_spmd

    x = np.asarray(x, np.float32)
    B, _, H, _ = x.shape
    BS = B // NCORES
    TYLO, TYHI = -2, 2
    nc = _get_nc(H=H, BS=BS)
    in_maps = _prep_host_inputs(
        x, np.asarray(weight), np.asarray(bias), np.asarray(offset_w),
        np.asarray(offset_b), np.asarray(mask_w), np.asarray(mask_b),
        H, BS, TYHI - TYLO + 1)
    res = run_bass_kernel_spmd(nc, in_maps, core_ids=list(range(NCORES)))
    outs = [res.results[i]["out"].reshape(BS, O, H, W) for i in range(NCORES)]
    return np.concatenate(outs, axis=0)


# revision 28
# speedup vs baseline: 1.1142x; 1.0803x over previous
"""DCNv2 (deformable conv) Trainium2 Bass kernel.

Strategy (per core, pure batch data-parallel across 8 cores):
  - x padded (+1) on host; SBUF resident per image: xs [96c, (H+2)(W+2)].
  - PE computes offset/mask 3x3 convs (9 accumulating matmuls per output row,
    pixels-on-partitions psum [128w, 27]) and per-tap 1x1 convs
    Y_k(o,r,w) = sum_c W[o,c,k] x(c,r,w) into a banded SBUF tensor
    YT [128w, (RB+6)rows, 9k, 96o] with zero row-halo.
  - DVE builds per-pixel bilinear row/col interpolation fields vy/hxm
    (integer sample-offset one-hots weighted by frac parts; mask folded
    into hxm).  Column shifts u are realized by DMA partition-shifted
    copies of vy/hxm (DMA is exempt from the partition-window rule).
  - DVE combine, per output row h and column shift u: one TT-mult with
    multi-dim APs over (i, j, ty, o) times q_u, then an XYZ tensor_reduce
    -> red_u[s, o] (s = source column).  PE merges the 7 shifted partials
    with shifted-identity matmuls accumulating psum[o, w]; ACT adds bias
    during the PSUM->SBUF copy; DMA out per band.
"""

import sys

sys.path.insert(0, "/opt/trn_rl_repo")

import numpy as np

import concourse.bacc as bacc
import concourse.bass as bass
import concourse.mybir as mybir
from concourse.tile import TileContext

F32 = mybir.dt.float32
AF = mybir.ActivationFunctionType
AL = mybir.AluOpType

C = 96
O = 96
NTAP = 9
W = 128
NCORES = 8


def build_nc(H=128, BS=2, RB=8, TYLO=-2, TYHI=2, num_devices=NCORES):
    """Build the per-core Bass program."""
    nty = TYHI - TYLO + 1
    ntx = nty
    TXLO, TXHI = TYLO, TYHI
    Hp, Wp = H + 2, W + 2
    RS = NTAP * O          # YT row stride = 864
    NROW = RB + 6          # YT band rows incl +-3 halo
    NK9 = RB * NTAP
    US = list(range(TXLO - 1, TXHI + 2))   # column shifts u = (j-1)+tx
    assert H % RB == 0

    nc = bacc.Bacc("TRN2", target_bir_lowering=False, debug=False,
                   num_devices=num_devices, dynamic_dma_scratch_size=2048)

    xp = nc.dram_tensor("xp", [BS, C, Hp * Wp], F32, kind="ExternalInput")
    wmain = nc.dram_tensor("wmain", [C, NTAP * O], F32, kind="ExternalInput")
    womb = nc.dram_tensor("womb", [C, NTAP * 27], F32, kind="ExternalInput")
    obrep = nc.dram_tensor("obrep", [W, 27], F32, kind="ExternalInput")
    ity = nc.dram_tensor("ity", [W, nty], F32, kind="ExternalInput")
    identw = nc.dram_tensor("identw", [W, W + 6], F32, kind="ExternalInput")
    biaso = nc.dram_tensor("biaso", [O, 1], F32, kind="ExternalInput")
    out = nc.dram_tensor("out", [BS, O, H * W], F32, kind="ExternalOutput")

    def sb_view(tile, offset, dims):
        return bass.AP(tensor=tile.tensor, offset=int(tile.offset) + offset,
                       ap=[list(d) for d in dims])

    with TileContext(nc) as tc:
        with (
            tc.tile_pool(name="consts", bufs=1) as cpool,
            tc.tile_pool(name="xs", bufs=2) as xpool,
            tc.tile_pool(name="yt", bufs=1) as ytpool,
            tc.tile_pool(name="fields", bufs=1) as fpool,
            tc.tile_pool(name="shf", bufs=1) as shpool,
            tc.tile_pool(name="qp", bufs=1) as qpool,
            tc.tile_pool(name="tmp", bufs=2) as tmppool,
            tc.tile_pool(name="small", bufs=4) as spool,
            tc.tile_pool(name="obuf", bufs=1) as opool,
            tc.tile_pool(name="psum_om", bufs=2, space="PSUM") as ompool,
            tc.tile_pool(name="psum_y", bufs=4, space="PSUM") as ypool,
            tc.tile_pool(name="psum_t", bufs=2, space="PSUM") as tpool,
        ):
            wmain_sb = cpool.tile([C, NTAP * O], F32)
            womb_sb = cpool.tile([C, NTAP * 27], F32)
            obrep_sb = cpool.tile([W, 27], F32)
            ity_sb = cpool.tile([W, nty], F32)
            identw_sb = cpool.tile([W, W + 6], F32)
            biaso_sb = cpool.tile([O, 1], F32)
            nc.sync.dma_start(wmain_sb[:], wmain[:])
            nc.sync.dma_start(womb_sb[:], womb[:])
            nc.sync.dma_start(obrep_sb[:], obrep[:])
            nc.sync.dma_start(ity_sb[:], ity[:])
            nc.sync.dma_start(identw_sb[:], identw[:])
            nc.sync.dma_start(biaso_sb[:], biaso[:])

            # partition-shifted field copies: value at partition s is the
            # field of destination column w = s - u; zero in the strips.
            vy_u = {}
            hxm_u = {}
            for u in US:
                if u == 0:
                    continue
                vy_u[u] = shpool.tile([W, NK9 * nty], F32, tag=f"vyu{u}", name=f"vyu{u}")
                hxm_u[u] = shpool.tile([W, NK9 * ntx], F32, tag=f"hxu{u}", name=f"hxu{u}")
                nc.vector.memset(vy_u[u][:], 0.0)
                nc.vector.memset(hxm_u[u][:], 0.0)

            XBROW = RB + 6  # band x rows: padded rows [b0-2, b0+RB+4)
            for img in range(BS):
                for band in range(H // RB):
                    b0 = band * RB
                    xs = xpool.tile([C, XBROW * Wp], F32, tag="xs")
                    rlo = max(0, b0 - 2)
                    rhi = min(Hp, b0 + RB + 4)
                    dst0 = (rlo - (b0 - 2)) * Wp
                    nc.sync.dma_start(
                        xs[:, dst0:dst0 + (rhi - rlo) * Wp],
                        bass.AP(tensor=xp,
                                offset=img * C * Hp * Wp + rlo * Wp,
                                ap=[[Hp * Wp, C], [1, (rhi - rlo) * Wp]]))

                    # ---- offset/mask convs + raw fields -------------------
                    raw = fpool.tile([W, RB * 27], F32, tag="raw")
                    for hh in range(RB):
                        h = b0 + hh
                        ps_om = ompool.tile([W, 27], F32, tag="om")
                        for t in range(NTAP):
                            ti, tj = t // 3, t % 3
                            lhsT = sb_view(xs, (hh + ti + 2) * Wp + tj,
                                           [[XBROW * Wp, C], [1, W]])
                            nc.tensor.matmul(ps_om[:], lhsT,
                                             womb_sb[:, t * 27:(t + 1) * 27],
                                             start=(t == 0), stop=(t == NTAP - 1))
                        nc.vector.tensor_add(
                            out=raw[:, hh * 27:(hh + 1) * 27],
                            in0=ps_om[:], in1=obrep_sb[:])

                    # ---- per-pixel interpolation fields -------------------
                    dyv = sb_view(raw, 0, [[RB * 27, W], [27, RB], [2, NTAP]])
                    dxv = sb_view(raw, 1, [[RB * 27, W], [27, RB], [2, NTAP]])
                    mrawv = sb_view(raw, 18, [[RB * 27, W], [27, RB], [1, NTAP]])

                    def pk2(tile):  # packed [hh][k] view matching (RB, NTAP)
                        return sb_view(tile, 0, [[NK9, W], [NTAP, RB], [1, NTAP]])

                    msk = fpool.tile([W, NK9], F32, tag="msk")
                    nc.scalar.activation(out=pk2(msk), in_=mrawv, func=AF.Sigmoid)

                    MAGIC = 12582912.0  # 1.5 * 2**23: fp32 round-to-int magic

                    def frac_int(dv, tag):
                        # v = dv + 2 in (0.2, 3.8); e2 = floor(v); fr = v - e2
                        tt = fpool.tile([W, NK9], F32, tag=f"T{tag}")
                        t3 = fpool.tile([W, NK9], F32, tag=f"t3{tag}")
                        fr = fpool.tile([W, NK9], F32, tag=f"f{tag}")
                        e2 = fpool.tile([W, NK9], F32, tag=f"e{tag}")
                        nc.vector.tensor_scalar(out=pk2(tt), in0=dv, scalar1=2.0,
                                                scalar2=None, op0=AL.add)
                        nc.vector.tensor_scalar(out=t3[:], in0=tt[:],
                                                scalar1=-0.5, scalar2=MAGIC,
                                                op0=AL.add, op1=AL.add)
                        nc.vector.tensor_scalar(out=e2[:], in0=t3[:],
                                                scalar1=-MAGIC, scalar2=None,
                                                op0=AL.add)
                        nc.vector.tensor_sub(out=fr[:], in0=tt[:], in1=e2[:])
                        return fr, e2  # frac, floor+2 (exact int-valued)

                    fy, ey2 = frac_int(dyv, "y")
                    fx, ex2 = frac_int(dxv, "x")

                    def eq_pair(e2, lo, tag):
                        c0 = fpool.tile([W, NK9], F32, tag=f"c0{tag}")
                        c1 = fpool.tile([W, NK9], F32, tag=f"c1{tag}")
                        nc.vector.tensor_scalar(out=c0[:], in0=e2[:],
                                                scalar1=float(-(2 + lo)),
                                                scalar2=None, op0=AL.add)
                        nc.vector.tensor_scalar(out=c1[:], in0=c0[:], scalar1=1.0,
                                                scalar2=None, op0=AL.add)
                        eq0 = fpool.tile([W, NK9 * nty], F32, tag=f"eq0{tag}")
                        eq1 = fpool.tile([W, NK9 * nty], F32, tag=f"eq1{tag}")
                        itv = sb_view(ity_sb, 0,
                                      [[nty, W], [0, RB], [0, NTAP], [1, nty]])
                        for eq, cc in ((eq0, c0), (eq1, c1)):
                            nc.vector.tensor_tensor(
                                out=sb_view(eq, 0, [[NK9 * nty, W],
                                                    [NTAP * nty, RB],
                                                    [nty, NTAP], [1, nty]]),
                                in0=itv,
                                in1=sb_view(cc, 0, [[NK9, W], [NTAP, RB],
                                                    [1, NTAP], [0, nty]]),
                                op=AL.is_equal)
                        return eq0, eq1

                    eq0y, eq1y = eq_pair(ey2, TYLO, "y")
                    eq0x, eq1x = eq_pair(ex2, TXLO, "x")

                    def lerp(eq0, eq1, w1, w0, tag, n):
                        # -> eq0*w0 + eq1*w1  ([W, NK9*n])
                        res = fpool.tile([W, NK9 * n], F32, tag=f"lp{tag}")
                        et = fpool.tile([W, NK9 * n], F32, tag="lerptmp")
                        bc = lambda t: sb_view(t, 0, [[NK9, W], [NTAP, RB],
                                                      [1, NTAP], [0, n]])
                        fl = lambda t: sb_view(t, 0, [[NK9 * n, W],
                                                      [NTAP * n, RB],
                                                      [n, NTAP], [1, n]])
                        nc.vector.tensor_tensor(out=fl(res), in0=fl(eq0),
                                                in1=bc(w0), op=AL.mult)
                        nc.vector.tensor_tensor(out=fl(et), in0=fl(eq1),
                                                in1=bc(w1), op=AL.mult)
                        nc.vector.tensor_add(out=res[:], in0=res[:], in1=et[:])
                        return res

                    fy1 = fpool.tile([W, NK9], F32, tag="fy1")
                    nc.vector.tensor_scalar(out=fy1[:], in0=fy[:], scalar1=-1.0,
                                            scalar2=1.0, op0=AL.mult, op1=AL.add)
                    vy = lerp(eq0y, eq1y, fy, fy1, "vy", nty)
                    fxm = fpool.tile([W, NK9], F32, tag="fxm")
                    fx1m = fpool.tile([W, NK9], F32, tag="fx1m")
                    nc.vector.tensor_mul(out=fxm[:], in0=fx[:], in1=msk[:])
                    nc.vector.tensor_sub(out=fx1m[:], in0=msk[:], in1=fxm[:])
                    hxm = lerp(eq0x, eq1x, fxm, fx1m, "hx", ntx)

                    # shifted copies via DMA (partition-window exempt)
                    for u in US:
                        if u == 0:
                            continue
                        cnt = W - abs(u)
                        dlo, slo = max(0, u), max(0, -u)
                        nc.sync.dma_start(vy_u[u][dlo:dlo + cnt, :],
                                          vy[slo:slo + cnt, :])
                        nc.sync.dma_start(hxm_u[u][dlo:dlo + cnt, :],
                                          hxm[slo:slo + cnt, :])

                    # q_u[s, (hh,i), j, ty] = vy_u * hxm_u(tx=u-(j-1))
                    q_u = {}
                    for u in US:
                        jlo = max(0, u + 1 - TXHI)
                        jhi = min(2, u + 1 - TXLO)
                        nj = jhi - jlo + 1
                        vyt = vy_u[u] if u else vy
                        hxt = hxm_u[u] if u else hxm
                        qt = qpool.tile([W, RB * 3 * nj * nty], F32,
                                        tag=f"q{u}", name=f"q{u}")
                        nc.vector.tensor_tensor(
                            out=sb_view(qt, 0, [[RB * 3 * nj * nty, W],
                                                [nj * nty, 3 * RB],
                                                [nty, nj], [1, nty]]),
                            in0=bass.AP(
                                tensor=vyt.tensor,
                                offset=int(vyt.offset) + jlo * nty,
                                ap=[[NK9 * nty, W], [3 * nty, 3 * RB],
                                    [nty, nj], [1, nty]]),
                            in1=bass.AP(
                                tensor=hxt.tensor,
                                offset=int(hxt.offset) + jlo * ntx
                                + (u - (jlo - 1) - TXLO),
                                ap=[[NK9 * ntx, W], [3 * ntx, 3 * RB],
                                    [ntx - 1, nj], [0, nty]]),
                            op=AL.mult)
                        q_u[u] = (qt, jlo, nj)

                    # ---- stage 1: per-tap 1x1 convs into YT band ----------
                    yt = ytpool.tile([W, NROW * RS], F32, tag="yt")
                    for rr in range(NROW):
                        r = b0 - 3 + rr
                        if r < 0 or r >= H:
                            nc.vector.memset(yt[:, rr * RS:(rr + 1) * RS], 0.0)
                            continue
                        for g in range(3):
                            ps_y = ypool.tile([W, 3 * O], F32, tag="y")
                            lhsT = sb_view(xs, rr * Wp + 1,
                                           [[XBROW * Wp, C], [1, W]])
                            nc.tensor.matmul(
                                ps_y[:], lhsT,
                                wmain_sb[:, g * 3 * O:(g + 1) * 3 * O],
                                start=True, stop=True)
                            nc.scalar.copy(
                                out=yt[:, rr * RS + g * 3 * O:
                                       rr * RS + (g + 1) * 3 * O],
                                in_=ps_y[:])

                    # ---- combine ------------------------------------------
                    obuf = opool.tile([O, RB * W], F32, tag="obuf")
                    UORD = [0, -2, -1, 2, 1, -3, 3] if len(US) == 7 else US
                    for hh in range(RB):
                        ps_t = tpool.tile([O, W], F32, tag="pt")
                        for iu, u in enumerate(UORD):
                            qt, jlo, nj = q_u[u]
                            yt_off = (hh + 2 + TYLO) * RS + jlo * O
                            # balance multiplies: GpSimd ~60%, DVE keeps reduces
                            on_gps = u not in (0, -1)
                            eng = nc.gpsimd if on_gps else nc.vector
                            tmp = tmppool.tile(
                                [W, 3 * 3 * nty * O], F32,
                                tag=("tmpg" if on_gps else "tmp"), name="tmp")
                            for i in range(3):
                                # in0: YT[s, row = hh+3+(i-1)+ty, k=(i,j), o]
                                in0 = bass.AP(
                                    tensor=yt.tensor,
                                    offset=int(yt.offset) + yt_off
                                    + i * (RS + 3 * O),
                                    ap=[[NROW * RS, W], [O, nj],
                                        [RS, nty], [1, O]])
                                in1 = bass.AP(
                                    tensor=qt.tensor,
                                    offset=int(qt.offset)
                                    + hh * (3 * nj * nty) + i * (nj * nty),
                                    ap=[[RB * 3 * nj * nty, W],
                                        [nty, nj], [1, nty], [0, O]])
                                tfree = 3 * 3 * nty * O
                                # tmp memory [o][i][j][ty]: reduce walk contiguous
                                tout = sb_view(tmp, i * (nj * nty),
                                               [[tfree, W],
                                                [nty, nj],
                                                [1, nty], [3 * nj * nty, O]])
                                eng.tensor_tensor(out=tout, in0=in0,
                                                  in1=in1, op=AL.mult)
                            tred = sb_view(tmp, 0,
                                           [[tfree, W], [3 * nj * nty, O],
                                            [1, 3 * nj * nty]])
                            red = spool.tile([W, O], F32, tag="red")
                            nc.vector.tensor_reduce(
                                out=red[:], in_=tred,
                                axis=mybir.AxisListType.X, op=AL.add)
                            # psum[o, w] += red[s, o] with w = s - u
                            rhs = sb_view(identw_sb, 3 + u,
                                          [[W + 6, W], [1, W]])
                            nc.tensor.matmul(ps_t[:], red[:, :O], rhs,
                                             start=(iu == 0),
                                             stop=(iu == len(UORD) - 1))
                        nc.scalar.activation(out=obuf[:, hh * W:(hh + 1) * W],
                                             in_=ps_t[:], func=AF.Identity,
                                             bias=biaso_sb[:], scale=1.0)

                    nc.sync.dma_start(
                        bass.AP(tensor=out,
                                offset=img * O * H * W + b0 * W,
                                ap=[[H * W, O], [1, RB * W]]),
                        obuf[:])

    nc.compile()
    return nc


# ---------------------------------------------------------------------------
def _prep_host_inputs(x, weight, bias, offset_w, offset_b, mask_w, mask_b,
                      H, BS, nty):
    """Build per-core input maps (host-side layout marshalling only)."""
    B = x.shape[0]
    Hp, Wp = H + 2, W + 2
    ncores = B // BS
    xp = np.zeros((B, C, Hp, Wp), np.float32)
    xp[:, :, 1:1 + H, 1:1 + W] = x
    xp = xp.reshape(B, C, Hp * Wp)

    wmain = np.ascontiguousarray(
        weight.transpose(1, 2, 3, 0).reshape(C, NTAP * O)).astype(np.float32)
    wo = offset_w.transpose(1, 2, 3, 0)   # [C, 3, 3, 18]
    wm = mask_w.transpose(1, 2, 3, 0)     # [C, 3, 3, 9]
    womb = np.concatenate([wo, wm], axis=3).reshape(C, NTAP * 27)
    womb = np.ascontiguousarray(womb).astype(np.float32)
    ob27 = np.concatenate([offset_b, mask_b]).astype(np.float32)
    obrep = np.broadcast_to(ob27, (W, 27)).copy()
    ity = np.broadcast_to(np.arange(nty, dtype=np.float32), (W, nty)).copy()
    identw = np.zeros((W, W + 6), np.float32)
    identw[np.arange(W), np.arange(W) + 3] = 1.0
    biaso = bias.astype(np.float32).reshape(O, 1)

    shared = dict(wmain=wmain, womb=womb, obrep=obrep, ity=ity,
                  identw=identw, biaso=biaso)
    in_maps = []
    for corei in range(ncores):
        m = dict(shared)
        m["xp"] = np.ascontiguousarray(xp[corei * BS:(corei + 1) * BS])
        in_maps.append(m)
    return in_maps


_NC_CACHE = {}


def _get_nc(H=128, BS=2, RB=8, TYLO=-2, TYHI=2):
    key = (H, BS, RB, TYLO, TYHI)
    if key not in _NC_CACHE:
        _NC_CACHE[key] = build_nc(H, BS, RB, TYLO, TYHI)
    return _NC_CACHE[key]


def kernel(x, weight, bias, offset_w, offset_b, mask_w, mask_b):
    from concourse.bass_utils import run_bass_kernel_spmd

    x = np.asarray(x, np.float32)
    B, _, H, _ = x.shape
    BS = B // NCORES
    TYLO, TYHI = -2, 2
    nc = _get_nc(H=H, BS=BS)
    in_maps = _prep_host_inputs(
        x, np.asarray(weight), np.asarray(bias), np.asarray(offset_w),
        np.asarray(offset_b), np.asarray(mask_w), np.asarray(mask_b),
        H, BS, TYHI - TYLO + 1)
    res = run_bass_kernel_spmd(nc, in_maps, core_ids=list(range(NCORES)))
    outs = [res.results[i]["out"].reshape(BS, O, H, W) for i in range(NCORES)]
    return np.concatenate(outs, axis=0)


# revision 30
# speedup vs baseline: 1.2225x; 1.0972x over previous
"""DCNv2 (deformable conv) Trainium2 Bass kernel.

Strategy (per core, pure batch data-parallel across 8 cores):
  - x padded (+1) on host; streamed per band: xs [96c, (RB+6)(W+2)],
    double-buffered so the DMA overlaps compute.
  - PE computes offset/mask 3x3 convs (9 accumulating matmuls per output row,
    pixels-on-partitions psum [128w, 27]) and per-tap 1x1 convs
    Y_k(o,r,w) = sum_c W[o,c,k] x(c,r,w) into a banded SBUF tensor
    YT [128w, (RB+6)rows, 9k, 96o] with zero row-halo.
  - DVE builds per-pixel bilinear row/col interpolation fields vy/hxm
    (integer sample-offset one-hots weighted by frac parts; mask folded
    into hxm).  Column shifts u are realized by DMA partition-shifted
    copies of vy/hxm (DMA is exempt from the partition-window rule).
  - Combine, per output row h and column shift u: TT-mults with multi-dim
    APs over (i, j, ty, o) times q_u (split across DVE [u in {0,-1}] and
    GpSimd [other u] for engine balance, interleaved through the chain),
    then a contiguous-innermost tensor_reduce -> red_u[s, o] (s = source
    column).  PE merges the 7 shifted partials with shifted-identity
    matmuls accumulating psum[o, w]; ACT adds bias during the PSUM->SBUF
    copy; DMA out per band.
"""

import sys

sys.path.insert(0, "/opt/trn_rl_repo")

import numpy as np

import concourse.bacc as bacc
import concourse.bass as bass
import concourse.mybir as mybir
from concourse.tile import TileContext

F32 = mybir.dt.float32
AF = mybir.ActivationFunctionType
AL = mybir.AluOpType

C = 96
O = 96
NTAP = 9
W = 128
NCORES = 8


def build_nc(H=128, BS=2, RB=8, TYLO=-2, TYHI=2, num_devices=NCORES):
    """Build the per-core Bass program."""
    nty = TYHI - TYLO + 1
    ntx = nty
    TXLO, TXHI = TYLO, TYHI
    Hp, Wp = H + 2, W + 2
    RS = NTAP * O          # YT row stride = 864
    NROW = RB + 6          # YT band rows incl +-3 halo
    NK9 = RB * NTAP
    US = list(range(TXLO - 1, TXHI + 2))   # column shifts u = (j-1)+tx
    assert H % RB == 0

    nc = bacc.Bacc("TRN2", target_bir_lowering=False, debug=False,
                   num_devices=num_devices, dynamic_dma_scratch_size=2048)

    xp = nc.dram_tensor("xp", [BS, C, Hp * Wp], F32, kind="ExternalInput")
    wmain = nc.dram_tensor("wmain", [C, NTAP * O], F32, kind="ExternalInput")
    womb = nc.dram_tensor("womb", [C, NTAP * 27], F32, kind="ExternalInput")
    obrep = nc.dram_tensor("obrep", [W, 27], F32, kind="ExternalInput")
    ity = nc.dram_tensor("ity", [W, nty], F32, kind="ExternalInput")
    identw = nc.dram_tensor("identw", [W, W + 6], F32, kind="ExternalInput")
    biaso = nc.dram_tensor("biaso", [O, 1], F32, kind="ExternalInput")
    out = nc.dram_tensor("out", [BS, O, H * W], F32, kind="ExternalOutput")

    def sb_view(tile, offset, dims):
        return bass.AP(tensor=tile.tensor, offset=int(tile.offset) + offset,
                       ap=[list(d) for d in dims])

    with TileContext(nc) as tc:
        with (
            tc.tile_pool(name="consts", bufs=1) as cpool,
            tc.tile_pool(name="xs", bufs=2) as xpool,
            tc.tile_pool(name="yt", bufs=1) as ytpool,
            tc.tile_pool(name="fields", bufs=1) as fpool,
            tc.tile_pool(name="shf", bufs=1) as shpool,
            tc.tile_pool(name="qp", bufs=1) as qpool,
            tc.tile_pool(name="tmp", bufs=2) as tmppool,
            tc.tile_pool(name="small", bufs=4) as spool,
            tc.tile_pool(name="obuf", bufs=1) as opool,
            tc.tile_pool(name="psum_om", bufs=2, space="PSUM") as ompool,
            tc.tile_pool(name="psum_y", bufs=4, space="PSUM") as ypool,
            tc.tile_pool(name="psum_t", bufs=2, space="PSUM") as tpool,
        ):
            wmain_sb = cpool.tile([C, NTAP * O], F32)
            womb_sb = cpool.tile([C, NTAP * 27], F32)
            obrep_sb = cpool.tile([W, 27], F32)
            ity_sb = cpool.tile([W, nty], F32)
            identw_sb = cpool.tile([W, W + 6], F32)
            biaso_sb = cpool.tile([O, 1], F32)
            nc.sync.dma_start(wmain_sb[:], wmain[:])
            nc.sync.dma_start(womb_sb[:], womb[:])
            nc.sync.dma_start(obrep_sb[:], obrep[:])
            nc.sync.dma_start(ity_sb[:], ity[:])
            nc.sync.dma_start(identw_sb[:], identw[:])
            nc.sync.dma_start(biaso_sb[:], biaso[:])

            # partition-shifted field copies: value at partition s is the
            # field of destination column w = s - u; zero in the strips.
            vy_u = {}
            hxm_u = {}
            for u in US:
                if u == 0:
                    continue
                vy_u[u] = shpool.tile([W, NK9 * nty], F32, tag=f"vyu{u}", name=f"vyu{u}")
                hxm_u[u] = shpool.tile([W, NK9 * ntx], F32, tag=f"hxu{u}", name=f"hxu{u}")
                nc.vector.memset(vy_u[u][:], 0.0)
                nc.vector.memset(hxm_u[u][:], 0.0)

            XBROW = RB + 6  # band x rows: padded rows [b0-2, b0+RB+4)
            for img in range(BS):
                for band in range(H // RB):
                    b0 = band * RB
                    xs = xpool.tile([C, XBROW * Wp], F32, tag="xs")
                    rlo = max(0, b0 - 2)
                    rhi = min(Hp, b0 + RB + 4)
                    dst0 = (rlo - (b0 - 2)) * Wp
                    nc.sync.dma_start(
                        xs[:, dst0:dst0 + (rhi - rlo) * Wp],
                        bass.AP(tensor=xp,
                                offset=img * C * Hp * Wp + rlo * Wp,
                                ap=[[Hp * Wp, C], [1, (rhi - rlo) * Wp]]))

                    # ---- offset/mask convs + raw fields -------------------
                    raw = fpool.tile([W, RB * 27], F32, tag="raw")
                    for hh in range(RB):
                        h = b0 + hh
                        ps_om = ompool.tile([W, 27], F32, tag="om")
                        for t in range(NTAP):
                            ti, tj = t // 3, t % 3
                            lhsT = sb_view(xs, (hh + ti + 2) * Wp + tj,
                                           [[XBROW * Wp, C], [1, W]])
                            nc.tensor.matmul(ps_om[:], lhsT,
                                             womb_sb[:, t * 27:(t + 1) * 27],
                                             start=(t == 0), stop=(t == NTAP - 1))
                        nc.vector.tensor_add(
                            out=raw[:, hh * 27:(hh + 1) * 27],
                            in0=ps_om[:], in1=obrep_sb[:])

                    # ---- per-pixel interpolation fields -------------------
                    dyv = sb_view(raw, 0, [[RB * 27, W], [27, RB], [2, NTAP]])
                    dxv = sb_view(raw, 1, [[RB * 27, W], [27, RB], [2, NTAP]])
                    mrawv = sb_view(raw, 18, [[RB * 27, W], [27, RB], [1, NTAP]])

                    def pk2(tile):  # packed [hh][k] view matching (RB, NTAP)
                        return sb_view(tile, 0, [[NK9, W], [NTAP, RB], [1, NTAP]])

                    msk = fpool.tile([W, NK9], F32, tag="msk")
                    nc.scalar.activation(out=pk2(msk), in_=mrawv, func=AF.Sigmoid)

                    MAGIC = 12582912.0  # 1.5 * 2**23: fp32 round-to-int magic

                    def frac_int(dv, tag):
                        # v = dv + 2 in (0.2, 3.8); e2 = floor(v); fr = v - e2
                        tt = fpool.tile([W, NK9], F32, tag=f"T{tag}")
                        t3 = fpool.tile([W, NK9], F32, tag=f"t3{tag}")
                        fr = fpool.tile([W, NK9], F32, tag=f"f{tag}")
                        e2 = fpool.tile([W, NK9], F32, tag=f"e{tag}")
                        nc.vector.tensor_scalar(out=pk2(tt), in0=dv, scalar1=2.0,
                                                scalar2=None, op0=AL.add)
                        nc.vector.tensor_scalar(out=t3[:], in0=tt[:],
                                                scalar1=-0.5, scalar2=MAGIC,
                                                op0=AL.add, op1=AL.add)
                        nc.vector.tensor_scalar(out=e2[:], in0=t3[:],
                                                scalar1=-MAGIC, scalar2=None,
                                                op0=AL.add)
                        nc.vector.tensor_sub(out=fr[:], in0=tt[:], in1=e2[:])
                        return fr, e2  # frac, floor+2 (exact int-valued)

                    fy, ey2 = frac_int(dyv, "y")
                    fx, ex2 = frac_int(dxv, "x")

                    def eq_pair(e2, lo, tag):
                        c0 = fpool.tile([W, NK9], F32, tag=f"c0{tag}")
                        c1 = fpool.tile([W, NK9], F32, tag=f"c1{tag}")
                        nc.vector.tensor_scalar(out=c0[:], in0=e2[:],
                                                scalar1=float(-(2 + lo)),
                                                scalar2=None, op0=AL.add)
                        nc.vector.tensor_scalar(out=c1[:], in0=c0[:], scalar1=1.0,
                                                scalar2=None, op0=AL.add)
                        eq0 = fpool.tile([W, NK9 * nty], F32, tag=f"eq0{tag}")
                        eq1 = fpool.tile([W, NK9 * nty], F32, tag=f"eq1{tag}")
                        itv = sb_view(ity_sb, 0,
                                      [[nty, W], [0, RB], [0, NTAP], [1, nty]])
                        for eq, cc in ((eq0, c0), (eq1, c1)):
                            nc.vector.tensor_tensor(
                                out=sb_view(eq, 0, [[NK9 * nty, W],
                                                    [NTAP * nty, RB],
                                                    [nty, NTAP], [1, nty]]),
                                in0=itv,
                                in1=sb_view(cc, 0, [[NK9, W], [NTAP, RB],
                                                    [1, NTAP], [0, nty]]),
                                op=AL.is_equal)
                        return eq0, eq1

                    eq0y, eq1y = eq_pair(ey2, TYLO, "y")
                    eq0x, eq1x = eq_pair(ex2, TXLO, "x")

                    def lerp(eq0, eq1, w1, w0, tag, n):
                        # -> eq0*w0 + eq1*w1  ([W, NK9*n])
                        res = fpool.tile([W, NK9 * n], F32, tag=f"lp{tag}")
                        et = fpool.tile([W, NK9 * n], F32, tag="lerptmp")
                        bc = lambda t: sb_view(t, 0, [[NK9, W], [NTAP, RB],
                                                      [1, NTAP], [0, n]])
                        fl = lambda t: sb_view(t, 0, [[NK9 * n, W],
                                                      [NTAP * n, RB],
                                                      [n, NTAP], [1, n]])
                        nc.vector.tensor_tensor(out=fl(res), in0=fl(eq0),
                                                in1=bc(w0), op=AL.mult)
                        nc.vector.tensor_tensor(out=fl(et), in0=fl(eq1),
                                                in1=bc(w1), op=AL.mult)
                        nc.vector.tensor_add(out=res[:], in0=res[:], in1=et[:])
                        return res

                    fy1 = fpool.tile([W, NK9], F32, tag="fy1")
                    nc.vector.tensor_scalar(out=fy1[:], in0=fy[:], scalar1=-1.0,
                                            scalar2=1.0, op0=AL.mult, op1=AL.add)
                    vy = lerp(eq0y, eq1y, fy, fy1, "vy", nty)
                    fxm = fpool.tile([W, NK9], F32, tag="fxm")
                    fx1m = fpool.tile([W, NK9], F32, tag="fx1m")
                    nc.vector.tensor_mul(out=fxm[:], in0=fx[:], in1=msk[:])
                    nc.vector.tensor_sub(out=fx1m[:], in0=msk[:], in1=fxm[:])
                    hxm = lerp(eq0x, eq1x, fxm, fx1m, "hx", ntx)

                    # shifted copies via DMA (partition-window exempt)
                    for u in US:
                        if u == 0:
                            continue
                        cnt = W - abs(u)
                        dlo, slo = max(0, u), max(0, -u)
                        nc.sync.dma_start(vy_u[u][dlo:dlo + cnt, :],
                                          vy[slo:slo + cnt, :])
                        nc.sync.dma_start(hxm_u[u][dlo:dlo + cnt, :],
                                          hxm[slo:slo + cnt, :])

                    # q_u[s, (hh,i), j, ty] = vy_u * hxm_u(tx=u-(j-1))
                    q_u = {}
                    for u in US:
                        jlo = max(0, u + 1 - TXHI)
                        jhi = min(2, u + 1 - TXLO)
                        nj = jhi - jlo + 1
                        vyt = vy_u[u] if u else vy
                        hxt = hxm_u[u] if u else hxm
                        qt = qpool.tile([W, RB * 3 * nj * nty], F32,
                                        tag=f"q{u}", name=f"q{u}")
                        nc.vector.tensor_tensor(
                            out=sb_view(qt, 0, [[RB * 3 * nj * nty, W],
                                                [nj * nty, 3 * RB],
                                                [nty, nj], [1, nty]]),
                            in0=bass.AP(
                                tensor=vyt.tensor,
                                offset=int(vyt.offset) + jlo * nty,
                                ap=[[NK9 * nty, W], [3 * nty, 3 * RB],
                                    [nty, nj], [1, nty]]),
                            in1=bass.AP(
                                tensor=hxt.tensor,
                                offset=int(hxt.offset) + jlo * ntx
                                + (u - (jlo - 1) - TXLO),
                                ap=[[NK9 * ntx, W], [3 * ntx, 3 * RB],
                                    [ntx - 1, nj], [0, nty]]),
                            op=AL.mult)
                        q_u[u] = (qt, jlo, nj)

                    # ---- stage 1: per-tap 1x1 convs into YT band ----------
                    yt = ytpool.tile([W, NROW * RS], F32, tag="yt")
                    for rr in range(NROW):
                        r = b0 - 3 + rr
                        if r < 0 or r >= H:
                            nc.vector.memset(yt[:, rr * RS:(rr + 1) * RS], 0.0)
                            continue
                        for g in range(3):
                            ps_y = ypool.tile([W, 3 * O], F32, tag="y")
                            lhsT = sb_view(xs, rr * Wp + 1,
                                           [[XBROW * Wp, C], [1, W]])
                            nc.tensor.matmul(
                                ps_y[:], lhsT,
                                wmain_sb[:, g * 3 * O:(g + 1) * 3 * O],
                                start=True, stop=True)
                            nc.scalar.copy(
                                out=yt[:, rr * RS + g * 3 * O:
                                       rr * RS + (g + 1) * 3 * O],
                                in_=ps_y[:])

                    # ---- combine ------------------------------------------
                    obuf = opool.tile([O, RB * W], F32, tag="obuf")
                    UORD = [0, -2, -1, 2, 1, -3, 3] if len(US) == 7 else US
                    for hh in range(RB):
                        ps_t = tpool.tile([O, W], F32, tag="pt")
                        for iu, u in enumerate(UORD):
                            qt, jlo, nj = q_u[u]
                            yt_off = (hh + 2 + TYLO) * RS + jlo * O
                            # balance multiplies: GpSimd ~60%, DVE keeps reduces
                            on_gps = u not in (0, -1)
                            eng = nc.gpsimd if on_gps else nc.vector
                            tmp = tmppool.tile(
                                [W, 3 * 3 * nty * O], F32,
                                tag=("tmpg" if on_gps else "tmp"), name="tmp")
                            for i in range(3):
                                # fine balance: u=-1,i=0 also on GpSimd
                                eng = nc.gpsimd if (on_gps or (u == -1 and i == 0))                                     else nc.vector
                                # in0: YT[s, row = hh+3+(i-1)+ty, k=(i,j), o]
                                in0 = bass.AP(
                                    tensor=yt.tensor,
                                    offset=int(yt.offset) + yt_off
                                    + i * (RS + 3 * O),
                                    ap=[[NROW * RS, W], [O, nj],
                                        [RS, nty], [1, O]])
                                in1 = bass.AP(
                                    tensor=qt.tensor,
                                    offset=int(qt.offset)
                                    + hh * (3 * nj * nty) + i * (nj * nty),
                                    ap=[[RB * 3 * nj * nty, W],
                                        [nty, nj], [1, nty], [0, O]])
                                tfree = 3 * 3 * nty * O
                                # tmp memory [o][i][j][ty]: reduce walk contiguous
                                tout = sb_view(tmp, i * (nj * nty),
                                               [[tfree, W],
                                                [nty, nj],
                                                [1, nty], [3 * nj * nty, O]])
                                eng.tensor_tensor(out=tout, in0=in0,
                                                  in1=in1, op=AL.mult)
                            tred = sb_view(tmp, 0,
                                           [[tfree, W], [3 * nj * nty, O],
                                            [1, 3 * nj * nty]])
                            red = spool.tile([W, O], F32, tag="red")
                            nc.vector.tensor_reduce(
                                out=red[:], in_=tred,
                                axis=mybir.AxisListType.X, op=AL.add)
                            # psum[o, w] += red[s, o] with w = s - u
                            rhs = sb_view(identw_sb, 3 + u,
                                          [[W + 6, W], [1, W]])
                            nc.tensor.matmul(ps_t[:], red[:, :O], rhs,
                                             start=(iu == 0),
                                             stop=(iu == len(UORD) - 1))
                        nc.scalar.activation(out=obuf[:, hh * W:(hh + 1) * W],
                                             in_=ps_t[:], func=AF.Identity,
                                             bias=biaso_sb[:], scale=1.0)

                    nc.sync.dma_start(
                        bass.AP(tensor=out,
                                offset=img * O * H * W + b0 * W,
                                ap=[[H * W, O], [1, RB * W]]),
                        obuf[:])

    nc.compile()
    return nc


# ---------------------------------------------------------------------------
def _prep_host_inputs(x, weight, bias, offset_w, offset_b, mask_w, mask_b,
                      H, BS, nty):
    """Build per-core input maps (host-side layout marshalling only)."""
    B = x.shape[0]
    Hp, Wp = H + 2, W + 2
    ncores = B // BS
    xp = np.zeros((B, C, Hp, Wp), np.float32)
    xp[:, :, 1:1 + H, 1:1 + W] = x
    xp = xp.reshape(B, C, Hp * Wp)

    wmain = np.ascontiguousarray(
        weight.transpose(1, 2, 3, 0).reshape(C, NTAP * O)).astype(np.float32)
    wo = offset_w.transpose(1, 2, 3, 0)   # [C, 3, 3, 18]
    wm = mask_w.transpose(1, 2, 3, 0)     # [C, 3, 3, 9]
    womb = np.concatenate([wo, wm], axis=3).reshape(C, NTAP * 27)
    womb = np.ascontiguousarray(womb).astype(np.float32)
    ob27 = np.concatenate([offset_b, mask_b]).astype(np.float32)
    obrep = np.broadcast_to(ob27, (W, 27)).copy()
    ity = np.broadcast_to(np.arange(nty, dtype=np.float32), (W, nty)).copy()
    identw = np.zeros((W, W + 6), np.float32)
    identw[np.arange(W), np.arange(W) + 3] = 1.0
    biaso = bias.astype(np.float32).reshape(O, 1)

    shared = dict(wmain=wmain, womb=womb, obrep=obrep, ity=ity,
                  identw=identw, biaso=biaso)
    in_maps = []
    for corei in range(ncores):
        m = dict(shared)
        m["xp"] = np.ascontiguousarray(xp[corei * BS:(corei + 1) * BS])
        in_maps.append(m)
    return in_maps


_NC_CACHE = {}


def _get_nc(H=128, BS=2, RB=8, TYLO=-2, TYHI=2):
    key = (H, BS, RB, TYLO, TYHI)
    if key not in _NC_CACHE:
        _NC_CACHE[key] = build_nc(H, BS, RB, TYLO, TYHI)
    return _NC_CACHE[key]


def kernel(x, weight, bias, offset_w, offset_b, mask_w, mask_b):
    from concourse.bass_utils import run_bass_kernel_spmd

    x = np.asarray(x, np.float32)
    B, _, H, _ = x.shape
    BS = B // NCORES
    TYLO, TYHI = -2, 2
    nc = _get_nc(H=H, BS=BS)
    in_maps = _prep_host_inputs(
        x, np.asarray(weight), np.asarray(bias), np.asarray(offset_w),
        np.asarray(offset_b), np.asarray(mask_w), np.asarray(mask_b),
        H, BS, TYHI - TYLO + 1)
    res = run_bass_kernel_spmd(nc, in_maps, core_ids=list(range(NCORES)))
    outs = [res.results[i]["out"].reshape(BS, O, H, W) for i in range(NCORES)]
    return np.concatenate(outs, axis=0)
